# revision 2
# baseline (speedup 1.0000x reference)
"""EvoMultiheadSelfAttention Trainium2 kernel (8 NeuronCores, SPMD), v3.

Sharding: core = (batch b, group of 4 heads). Per core: project q/k/v for
its 4 heads, causal full attention + 64-wide sliding-window attention,
combine with sigmoid(gate), partial output projection over its 256-dim
d-slice. Host sums 4 partials per batch and adds bo.

Key structure:
  - Scores transposed sT[j, i] (keys on partitions); AV produces O[i, d]
    (lhsT = p tiles, rhs = v natural [t, d] + ones lane -> per-partition
    softmax denominators; normalization via per-partition tensor_scalar).
  - v projected DIRECTLY into [t, d] layout (lhsT = xT): no transposes.
  - q/k projections in fp8e4m3 + DoubleRow (2x PE throughput); v path
    stays bf16 (v quantization error dominates otherwise).
  - Causal mask on diag tiles via an extra PE matmul writing -30000 into
    PSUM before the score matmul (group kept contiguous - an intervening
    start=True on a bank wipes open accumulation groups).
  - Bulk score pairs in one 2-bank PSUM tile ([128,1024] exp ops); band
    tiles paired (0,3)/(1,2) into [128,640] exp ops.
  - Window masks multiplied on the Pool engine (SBUF-only, otherwise idle).
  - O transposed to oT[d, t] by PE transpose (f32); outproj per chunk;
    psum->sbuf out copies split DVE/Act; stores via SP DMA.
"""

import os
import numpy as np
import ml_dtypes

B, T, DM, H, WIN = 2, 2048, 1024, 16, 64
DH = DM // H          # 64
NCORES = 8
KS = DM // 128        # 8 d-subtiles
NT = T // 128         # 16 j/i tiles
NCH = T // 512        # 4 chunks of 512
BF16 = ml_dtypes.bfloat16
FP8 = ml_dtypes.float8_e4m3

STAGE = os.environ.get("EVO_STAGE", "B2")

_CACHE: dict = {}
SG = [0.11920292202211755]  # sigmoid(gate); set by _prep_inputs before build


def _build_module():
    import contextlib
    import concourse.bass as bass  # noqa: F401
    import concourse.mybir as mybir
    import concourse.tile as tile
    from concourse import bacc
    from concourse.bass import ts

    f32 = mybir.dt.float32
    bf16 = mybir.dt.bfloat16
    fp8 = mybir.dt.float8e4
    EXP = mybir.ActivationFunctionType.Exp
    IDENT = mybir.ActivationFunctionType.Identity
    MULT = mybir.AluOpType.mult
    ADD = mybir.AluOpType.add
    DR = mybir.MatmulPerfMode.DoubleRow

    fp8_qk = STAGE in ("B2",)
    qkdt = fp8 if fp8_qk else bf16
    qcs = (1.0 / 64.0) if fp8_qk else 1.0   # q copy scale (descale + 1/sqrt(dh))
    kcs = (1.0 / 8.0) if fp8_qk else 1.0    # k copy scale

    nc = bacc.Bacc("TRN2", target_bir_lowering=False, debug=False, num_devices=NCORES)

    def din(name, shape, dt):
        return nc.dram_tensor(name, shape, dt, kind="ExternalInput").ap()

    xTd = din("xTd", [NCH, 128, KS, 512], bf16)    # x[b]^T chunked (bf16, v path)
    x8d = din("x8d", [NCH, 128, KS, 512], qkdt)    # fp8 copy for q/k path
    wqk = din("wqk", [128, 2, 2, KS, 128], qkdt)   # [dp, q/k, p, ks, e']
    wvT = din("wvT", [128, KS, 256], bf16)
    wo = din("wo", [128, 2, DM], bf16)
    bqk = din("bqk", [128, 4], f32)                # [bq p0, bq p1, bk p0, bk p1]
    bvT = din("bvT", [1, 256], bf16)
    # masks packed: [negI | Utri | wm_sub | wm_diag]
    msk = din("msk", [128, 512], bf16)
    idf = din("idf", [128, 128], f32)              # identity for PE transpose
    out = nc.dram_tensor("out", [DM, T], bf16, kind="ExternalOutput").ap()

    with tile.TileContext(nc) as tc:
        ctx = contextlib.ExitStack()
        consts = ctx.enter_context(tc.tile_pool(name="consts", bufs=1))
        big = ctx.enter_context(tc.tile_pool(name="big", bufs=1))
        pbulk = ctx.enter_context(tc.tile_pool(name="pbulk", bufs=9))
        pband = ctx.enter_context(tc.tile_pool(name="pband", bufs=4))
        ppw = ctx.enter_context(tc.tile_pool(name="ppw", bufs=6))
        npool = ctx.enter_context(tc.tile_pool(name="npool", bufs=4))
        ochnk = ctx.enter_context(tc.tile_pool(name="ochnk", bufs=2))
        opool = ctx.enter_context(tc.tile_pool(name="opool", bufs=4))
        psS = ctx.enter_context(tc.tile_pool(name="psS", bufs=2, space="PSUM"))
        psAV = ctx.enter_context(tc.tile_pool(name="psAV", bufs=2, space="PSUM"))
        psPO = ctx.enter_context(tc.tile_pool(name="psPO", bufs=2, space="PSUM"))

        def cload(ap_in, shape, dt, tag):
            t_ = consts.tile(shape, dt, tag=tag, name=tag)
            nc.sync.dma_start(out=t_, in_=ap_in)
            return t_

        # critical-path loads first
        wqk_sb = cload(wqk, [128, 2, 2, KS, 128], qkdt, "wqk")
        bqk_sb = cload(bqk, [128, 4], f32, "bqk")
        msk_sb = cload(msk, [128, 512], bf16, "msk")
        negI_sb = msk_sb[:, 0:128]
        Utri_sb = msk_sb[:, 128:256]
        wm_sb = msk_sb[:, 256:512]

        # xT [dp, ks, t]
        xT = big.tile([128, KS, T], bf16, tag="xT", name="xT")
        x8 = (big.tile([128, KS, T], qkdt, tag="x8", name="x8")
              if fp8_qk else xT)
        qT = [big.tile([128, T], bf16, tag=f"qT{p}", name=f"qT{p}") for p in (0, 1)]
        kT = [big.tile([128, T], bf16, tag=f"kT{p}", name=f"kT{p}") for p in (0, 1)]
        vh = big.tile([128, NT, 4, 65], bf16, tag="vh", name="vh")
        nc.vector.memset(vh, 1.0)
        oT = big.tile([128, 2, T], bf16, tag="oT", name="oT")

        state = {"oc": None}

        def proj_qk(c4):
            if fp8_qk:
                nc.sync.dma_start(out=x8[:, :, ts(c4, 512)], in_=x8d[c4])
            for p in (0, 1):
                for qk, dst, cs in ((0, qT[p], qcs), (1, kT[p], kcs)):
                    w_sb = wqk_sb[:, qk, p]
                    b_sb = bqk_sb[:, 2 * qk + p:2 * qk + p + 1]
                    ps = psPO.tile([128, 512], f32, tag="po", name="po")
                    if fp8_qk:
                        for kk in range(4):
                            nc.tensor.matmul(ps, lhsT=w_sb[:, 2 * kk:2 * kk + 2, :],
                                             rhs=x8[:, 2 * kk:2 * kk + 2, ts(c4, 512)],
                                             perf_mode=DR,
                                             start=(kk == 0), stop=(kk == 3))
                    else:
                        for kk in range(KS):
                            nc.tensor.matmul(ps, lhsT=w_sb[:, kk, :],
                                             rhs=x8[:, kk, ts(c4, 512)],
                                             start=(kk == 0), stop=(kk == KS - 1))
                    nc.scalar.activation(dst[:, ts(c4, 512)], ps, IDENT,
                                         bias=b_sb, scale=cs)

        def proj_v(c4):
            nc.sync.dma_start(out=xT[:, :, ts(c4, 512)], in_=xTd[c4])
            for tt in range(4 * c4, 4 * c4 + 4):
                ps = psPO.tile([128, 512], f32, tag="po", name="po")
                pv = ps[:, 0:256]
                nc.tensor.matmul(pv, lhsT=onesrow, rhs=bvT_sb,
                                 start=True, stop=False, skip_group_check=True)
                for kk in range(KS):
                    nc.tensor.matmul(pv, lhsT=xT[:, kk, ts(tt, 128)],
                                     rhs=wvT_sb[:, kk, :],
                                     start=False, stop=(kk == KS - 1),
                                     skip_group_check=True)
                nc.vector.tensor_copy(vh[:, tt, :, 0:64], pv)

        def attn(c, h4):
            """Attention for query chunk c (512 queries), head h4 (0..3)."""
            p, hh = divmod(h4, 2)
            hb = 64 * hh
            kTl, qTl = kT[p], qT[p]
            nbulk = 4 * c
            pa = {}   # bulk pair tiles: pa[jp] covers jt = 2jp, 2jp+1
            for jp in range(nbulk // 2):
                ps = psS.tile([128, 1024], f32, tag="s", name="s")
                for q2 in (0, 1):
                    jt = 2 * jp + q2
                    nc.tensor.matmul(ps[:, ts(q2, 512)],
                                     lhsT=kTl[hb:hb + 64, ts(jt, 128)],
                                     rhs=qTl[hb:hb + 64, ts(c, 512)],
                                     start=True, stop=True,
                                     skip_group_check=(q2 == 1))
                t_ = pbulk.tile([128, 1024], bf16, tag="pa", name="pa")
                nc.scalar.activation(t_, ps, EXP)
                pa[jp] = t_
            # band: subtile m covers j-tiles 4c..4c+m; pairs (0,3), (1,2)
            pb = {}
            for mpair in ((0, 3), (1, 2)):
                ps = psS.tile([128, 1024], f32, tag="s", name="s")
                off = 0
                offs = {}
                for m in mpair:
                    t_ = 4 * c + m
                    for mm in range(m):
                        nc.tensor.matmul(ps[:, off + 128 * mm:off + 128 * mm + 128],
                                         lhsT=kTl[hb:hb + 64, ts(4 * c + mm, 128)],
                                         rhs=qTl[hb:hb + 64, ts(t_, 128)],
                                         start=True, stop=True,
                                         skip_group_check=True)
                    dg = slice(off + 128 * m, off + 128 * m + 128)
                    nc.tensor.matmul(ps[:, dg], lhsT=negI_sb, rhs=Utri_sb,
                                     start=True, stop=False, skip_group_check=True)
                    nc.tensor.matmul(ps[:, dg],
                                     lhsT=kTl[hb:hb + 64, ts(t_, 128)],
                                     rhs=qTl[hb:hb + 64, ts(t_, 128)],
                                     start=False, stop=True,
                                     skip_group_check=True)
                    offs[m] = off
                    off += (m + 1) * 128
                pbt = pband.tile([128, 640], bf16, tag="pb", name="pb")
                nc.scalar.activation(pbt[:, 0:off], ps[:, 0:off], EXP)
                for m in mpair:
                    pb[m] = pbt[:, offs[m]:offs[m] + (m + 1) * 128]
            # window masked probabilities (Pool engine: SBUF-only)
            pw = {}
            for m in range(4):
                t_ = 4 * c + m
                t2 = ppw.tile([128, 256], bf16, tag="pw", name="pw")
                if t_ > 0:
                    if m == 0:
                        sub_src = pa[nbulk // 2 - 1][:, 512:640]
                    else:
                        sub_src = pb[m][:, ts(m - 1, 128)]
                    nc.gpsimd.tensor_tensor(t2[:, 0:128], sub_src,
                                            wm_sb[:, 0:128], MULT)
                nc.gpsimd.tensor_tensor(t2[:, 128:256], pb[m][:, ts(m, 128)],
                                        wm_sb[:, 128:256], MULT)
                pw[m] = t2
            # AV + normalization, two i-subtiles per psum tile
            o_chunk = state["oc"]
            for mh in (0, 1):
                av = psAV.tile([128, 260], f32, tag="av", name="av")
                for mq in (0, 1):
                    m = 2 * mh + mq
                    t_ = 4 * c + m
                    base = 130 * mq
                    OF = av[:, base:base + 65]
                    OW = av[:, base + 65:base + 130]
                    first = True
                    for jt in range(nbulk):
                        nc.tensor.matmul(
                            OF,
                            lhsT=pa[jt // 2][:, 512 * (jt % 2) + 128 * m:
                                             512 * (jt % 2) + 128 * m + 128],
                            rhs=vh[:, jt, h4, :],
                            start=first, stop=False, skip_group_check=True)
                        first = False
                    for mm in range(m + 1):
                        jt = 4 * c + mm
                        nc.tensor.matmul(OF, lhsT=pb[m][:, ts(mm, 128)],
                                         rhs=vh[:, jt, h4, :],
                                         start=first, stop=(mm == m),
                                         skip_group_check=True)
                        first = False
                    if t_ > 0:
                        nc.tensor.matmul(OW, lhsT=pw[m][:, 0:128],
                                         rhs=vh[:, t_ - 1, h4, :],
                                         start=True, stop=False,
                                         skip_group_check=True)
                    nc.tensor.matmul(OW, lhsT=pw[m][:, 128:256],
                                     rhs=vh[:, t_, h4, :],
                                     start=(t_ == 0), stop=True,
                                     skip_group_check=True)
                rcp = npool.tile([128, 4], f32, tag="rcp", name="rcp")
                nc.vector.reciprocal(rcp, av[:, 64::65])
                rsg = npool.tile([128, 2], f32, tag="rsg", name="rsg")
                nc.vector.tensor_scalar(rsg, rcp[:, 1::2], float(SG[0]), None, MULT)
                for mq in (0, 1):
                    m = 2 * mh + mq
                    base = 130 * mq
                    tf = npool.tile([128, 64], f32, tag="tf", name="tf")
                    nc.vector.tensor_scalar(tf, av[:, base:base + 64],
                                            rcp[:, 2 * mq:2 * mq + 1], None, MULT)
                    nc.vector.scalar_tensor_tensor(
                        o_chunk[:, m, 64 * h4:64 * h4 + 64],
                        av[:, base + 65:base + 129],
                        rsg[:, mq:mq + 1], tf, MULT, ADD)

        def transp_outproj(c):
            o_chunk = state["oc"]
            for dhalf in (0, 1):
                pst = psPO.tile([128, 512], f32, tag="po", name="po")
                for m in range(4):
                    nc.tensor.matmul(pst[:, ts(m, 128)],
                                     lhsT=o_chunk[:, m, ts(dhalf, 128)],
                                     rhs=idf_sb, is_transpose=True,
                                     start=True, stop=True,
                                     skip_group_check=True)
                nc.vector.tensor_copy(oT[:, dhalf, ts(c, 512)], pst)
            for et in range(8):
                ps = psPO.tile([128, 512], f32, tag="po", name="po")
                for kk in (0, 1):
                    nc.tensor.matmul(ps, lhsT=wo_sb[:, kk, ts(et, 128)],
                                     rhs=oT[:, kk, ts(c, 512)],
                                     start=(kk == 0), stop=(kk == 1))
                ob = opool.tile([128, 512], bf16, tag="ob", name="ob")
                if et % 2 == 0:
                    nc.vector.tensor_copy(ob, ps)
                else:
                    nc.scalar.activation(ob, ps, IDENT)
                nc.sync.dma_start(out=out[ts(et, 128), ts(c, 512)], in_=ob)

        # q/k for chunk 0 start as soon as their weights + x8 arrive
        proj_qk(0)
        # remaining (non-critical-path) const loads
        wvT_sb = cload(wvT, [128, KS, 256], bf16, "wvT")
        bvT_blk = consts.tile([128, 256], bf16, tag="bvT", name="bvT")
        nc.sync.dma_start(out=bvT_blk[0:1, :], in_=bvT)
        bvT_sb = bvT_blk[0:1, :]
        ones_blk = consts.tile([128, 128], bf16, tag="ones", name="ones")
        nc.vector.memset(ones_blk, 1.0)
        onesrow = ones_blk[0:1, :]
        wo_sb = cload(wo, [128, 2, DM], bf16, "wo")
        idf_sb = cload(idf, [128, 128], f32, "idf")
        proj_v(0)
        for c in range(NCH):
            state["oc"] = ochnk.tile([128, 4, 256], f32, tag="oc", name="oc")
            for h4 in range(4):
                attn(c, h4)
                if h4 == 0 and c + 1 < NCH:
                    proj_qk(c + 1)
                    proj_v(c + 1)
            transp_outproj(c)
        ctx.close()

    nc.compile()
    return nc


def _get_module():
    if "nc" not in _CACHE:
        _CACHE["nc"] = _build_module()
    return _CACHE["nc"]


def _prep_inputs(x, Wq, bq, Wk, bk, Wv, bv, Wo, bo, gate):
    x = np.asarray(x, np.float32)
    Wq = np.asarray(Wq, np.float32)
    Wk = np.asarray(Wk, np.float32)
    Wv = np.asarray(Wv, np.float32)
    Wo = np.asarray(Wo, np.float32)
    bq = np.asarray(bq, np.float32)
    bk = np.asarray(bk, np.float32)
    bv = np.asarray(bv, np.float32)
    SG[0] = float(1.0 / (1.0 + np.exp(-np.float32(gate))))

    fp8_qk = STAGE in ("B2",)
    qknp = FP8 if fp8_qk else BF16
    wscale = 8.0 if fp8_qk else 1.0                       # k weight scale
    qwscale = 8.0 if fp8_qk else 1.0 / np.sqrt(np.float32(DH))
    qbscale = 1.0 / np.sqrt(np.float32(DH))

    j = np.arange(128)[:, None]
    i = np.arange(128)[None, :]
    negI = (-30000.0 * (j == i)).astype(BF16)
    Utri = (j > i).astype(BF16)
    wm_sub = (j >= i + 65).astype(BF16)
    wm_diag = ((j <= i) & (j >= i - 63)).astype(BF16)
    msk = np.concatenate([negI, Utri, wm_sub, wm_diag], axis=1)
    idf = np.eye(128, dtype=np.float32)

    def wslice(Wmat, e0, scl, dt):
        Ws = (Wmat[e0:e0 + 128, :] * scl).astype(np.float32)  # [128 e', DM d]
        return np.ascontiguousarray(
            Ws.T.reshape(KS, 128, 128).transpose(1, 0, 2)).astype(dt)

    in_maps = []
    for core in range(NCORES):
        b, g = divmod(core, 4)
        e0 = g * 256
        xT_full = np.ascontiguousarray(x[b].T)            # [DM, T]
        xTc = np.ascontiguousarray(
            xT_full.reshape(KS, 128, NCH, 512).transpose(2, 1, 0, 3))
        wq_c = np.stack([wslice(Wq, e0 + 128 * p, qwscale, qknp) for p in (0, 1)])
        wk_c = np.stack([wslice(Wk, e0 + 128 * p, wscale, qknp) for p in (0, 1)])
        Wvs = Wv[e0:e0 + 256, :].astype(np.float32)       # [256 e', DM]
        wvT_c = np.ascontiguousarray(
            Wvs.T.reshape(KS, 128, 256).transpose(1, 0, 2)).astype(BF16)
        Wos = Wo[:, e0:e0 + 256]                          # [DM e, 256 d]
        wo_c = np.ascontiguousarray(
            Wos.T.reshape(2, 128, DM).transpose(1, 0, 2)).astype(BF16)
        bqk_c = np.stack([bq[e0:e0 + 128] * qbscale,
                          bq[e0 + 128:e0 + 256] * qbscale,
                          bk[e0:e0 + 128], bk[e0 + 128:e0 + 256]],
                         axis=1).astype(np.float32)       # [128, 4]
        in_maps.append({
            "xTd": xTc.astype(BF16), "x8d": xTc.astype(qknp),
            "wqk": np.ascontiguousarray(np.stack([wq_c, wk_c]).transpose(2, 0, 1, 3, 4)), "wvT": wvT_c, "wo": wo_c,
            "bqk": bqk_c,
            "bvT": bv[e0:e0 + 256].reshape(1, 256).astype(BF16),
            "msk": msk, "idf": idf,
        })
    return in_maps


def _run(nc, in_maps, **kw):
    from concourse.bass_utils import run_bass_kernel_spmd
    from concourse.bass_interp import get_hw_module
    old = nc.m
    nc.m = get_hw_module(nc.m)
    try:
        res = run_bass_kernel_spmd(nc, in_maps, core_ids=list(range(NCORES)), **kw)
    finally:
        nc.m = old
    return res


def kernel(x, Wq, bq, Wk, bk, Wv, bv, Wo, bo, gate):
    in_maps = _prep_inputs(x, Wq, bq, Wk, bk, Wv, bv, Wo, bo, gate)
    nc = _get_module()
    res = _run(nc, in_maps)
    bo = np.asarray(bo, np.float32)
    out = np.zeros((B, T, DM), np.float32)
    for core in range(NCORES):
        b = core // 4
        out[b] += res.results[core]["out"].astype(np.float32).T
    out += bo[None, None, :]
    return out


# revision 3
# speedup vs baseline: 1.0237x; 1.0237x over previous
"""EvoMultiheadSelfAttention Trainium2 kernel (8 NeuronCores, SPMD), v3.

Sharding: core = (batch b, group of 4 heads). Per core: project q/k/v for
its 4 heads, causal full attention + 64-wide sliding-window attention,
combine with sigmoid(gate), partial output projection over its 256-dim
d-slice. Host sums 4 partials per batch and adds bo.

Key structure:
  - Scores transposed sT[j, i] (keys on partitions); AV produces O[i, d]
    (lhsT = p tiles, rhs = v natural [t, d] + ones lane -> per-partition
    softmax denominators; normalization via per-partition tensor_scalar).
  - v projected DIRECTLY into [t, d] layout (lhsT = xT): no transposes.
  - q/k projections in fp8e4m3 + DoubleRow (2x PE throughput); v path
    stays bf16 (v quantization error dominates otherwise).
  - Causal mask on diag tiles via an extra PE matmul writing -30000 into
    PSUM before the score matmul (group kept contiguous - an intervening
    start=True on a bank wipes open accumulation groups).
  - Bulk score pairs in one 2-bank PSUM tile ([128,1024] exp ops); band
    tiles paired (0,3)/(1,2) into [128,640] exp ops.
  - Window masks multiplied on the Pool engine (SBUF-only, otherwise idle).
  - O transposed to oT[d, t] by PE transpose (f32); outproj per chunk;
    psum->sbuf out copies split DVE/Act; stores via SP DMA.
"""

import os
import numpy as np
import ml_dtypes

B, T, DM, H, WIN = 2, 2048, 1024, 16, 64
DH = DM // H          # 64
NCORES = 8
KS = DM // 128        # 8 d-subtiles
NT = T // 128         # 16 j/i tiles
NCH = T // 512        # 4 chunks of 512
BF16 = ml_dtypes.bfloat16
FP8 = ml_dtypes.float8_e4m3

STAGE = os.environ.get("EVO_STAGE", "B2")

_CACHE: dict = {}
SG = [0.11920292202211755]  # sigmoid(gate); set by _prep_inputs before build


def _build_module():
    import contextlib
    import concourse.bass as bass  # noqa: F401
    import concourse.mybir as mybir
    import concourse.tile as tile
    from concourse import bacc
    from concourse.bass import ts

    f32 = mybir.dt.float32
    bf16 = mybir.dt.bfloat16
    fp8 = mybir.dt.float8e4
    EXP = mybir.ActivationFunctionType.Exp
    IDENT = mybir.ActivationFunctionType.Identity
    MULT = mybir.AluOpType.mult
    ADD = mybir.AluOpType.add
    DR = mybir.MatmulPerfMode.DoubleRow

    fp8_qk = STAGE in ("B2",)
    qkdt = fp8 if fp8_qk else bf16
    qcs = (1.0 / 64.0) if fp8_qk else 1.0   # q copy scale (descale + 1/sqrt(dh))
    kcs = (1.0 / 8.0) if fp8_qk else 1.0    # k copy scale

    nc = bacc.Bacc("TRN2", target_bir_lowering=False, debug=False, num_devices=NCORES)

    def din(name, shape, dt):
        return nc.dram_tensor(name, shape, dt, kind="ExternalInput").ap()

    xTd = din("xTd", [NCH, 128, KS, 512], bf16)    # x[b]^T chunked (bf16, v path)
    x8d = din("x8d", [NCH, 128, KS, 512], qkdt)    # fp8 copy for q/k path
    wqk = din("wqk", [128, 2, 2, KS, 128], qkdt)   # [dp, q/k, p, ks, e']
    wvT = din("wvT", [128, KS, 256], bf16)
    wo = din("wo", [128, 2, DM], bf16)
    bqk = din("bqk", [128, 4], f32)                # [bq p0, bq p1, bk p0, bk p1]
    bvT = din("bvT", [1, 256], bf16)
    # masks packed: [negI | Utri | wm_sub | wm_diag]
    msk = din("msk", [128, 512], bf16)
    idf = din("idf", [128, 128], f32)              # identity for PE transpose
    out = nc.dram_tensor("out", [DM, T], bf16, kind="ExternalOutput").ap()

    with tile.TileContext(nc) as tc:
        ctx = contextlib.ExitStack()
        consts = ctx.enter_context(tc.tile_pool(name="consts", bufs=1))
        big = ctx.enter_context(tc.tile_pool(name="big", bufs=1))
        pbulk = ctx.enter_context(tc.tile_pool(name="pbulk", bufs=14))
        pband = ctx.enter_context(tc.tile_pool(name="pband", bufs=6))
        ppw = ctx.enter_context(tc.tile_pool(name="ppw", bufs=9))
        npool = ctx.enter_context(tc.tile_pool(name="npool", bufs=4))
        ochnk = ctx.enter_context(tc.tile_pool(name="ochnk", bufs=2))
        opool = ctx.enter_context(tc.tile_pool(name="opool", bufs=6))
        psS = ctx.enter_context(tc.tile_pool(name="psS", bufs=2, space="PSUM"))
        psAV = ctx.enter_context(tc.tile_pool(name="psAV", bufs=2, space="PSUM"))
        psPO = ctx.enter_context(tc.tile_pool(name="psPO", bufs=2, space="PSUM"))

        def cload(ap_in, shape, dt, tag):
            t_ = consts.tile(shape, dt, tag=tag, name=tag)
            nc.sync.dma_start(out=t_, in_=ap_in)
            return t_

        # critical-path loads first
        wqk_sb = cload(wqk, [128, 2, 2, KS, 128], qkdt, "wqk")
        bqk_sb = cload(bqk, [128, 4], f32, "bqk")
        msk_sb = cload(msk, [128, 512], bf16, "msk")
        negI_sb = msk_sb[:, 0:128]
        Utri_sb = msk_sb[:, 128:256]
        wm_sb = msk_sb[:, 256:512]

        # xT [dp, ks, t]
        xT = big.tile([128, KS, T], bf16, tag="xT", name="xT")
        x8 = (big.tile([128, KS, T], qkdt, tag="x8", name="x8")
              if fp8_qk else xT)
        qT = [big.tile([128, T], bf16, tag=f"qT{p}", name=f"qT{p}") for p in (0, 1)]
        kT = [big.tile([128, T], bf16, tag=f"kT{p}", name=f"kT{p}") for p in (0, 1)]
        vh = big.tile([128, NT, 4, 65], bf16, tag="vh", name="vh")
        nc.vector.memset(vh, 1.0)
        oT = big.tile([128, 2, T], bf16, tag="oT", name="oT")

        state = {"oc": None}

        def proj_qk(c4):
            if fp8_qk:
                nc.sync.dma_start(out=x8[:, :, ts(c4, 512)], in_=x8d[c4])
            for p in (0, 1):
                for qk, dst, cs in ((0, qT[p], qcs), (1, kT[p], kcs)):
                    w_sb = wqk_sb[:, qk, p]
                    b_sb = bqk_sb[:, 2 * qk + p:2 * qk + p + 1]
                    ps = psPO.tile([128, 512], f32, tag="po", name="po")
                    if fp8_qk:
                        for kk in range(4):
                            nc.tensor.matmul(ps, lhsT=w_sb[:, 2 * kk:2 * kk + 2, :],
                                             rhs=x8[:, 2 * kk:2 * kk + 2, ts(c4, 512)],
                                             perf_mode=DR,
                                             start=(kk == 0), stop=(kk == 3))
                    else:
                        for kk in range(KS):
                            nc.tensor.matmul(ps, lhsT=w_sb[:, kk, :],
                                             rhs=x8[:, kk, ts(c4, 512)],
                                             start=(kk == 0), stop=(kk == KS - 1))
                    nc.vector.tensor_scalar(dst[:, ts(c4, 512)], ps, cs, b_sb,
                                            MULT, ADD)

        def proj_v(c4):
            nc.sync.dma_start(out=xT[:, :, ts(c4, 512)], in_=xTd[c4])
            for tt in range(4 * c4, 4 * c4 + 4):
                ps = psPO.tile([128, 512], f32, tag="po", name="po")
                pv = ps[:, 0:256]
                nc.tensor.matmul(pv, lhsT=onesrow, rhs=bvT_sb,
                                 start=True, stop=False, skip_group_check=True)
                for kk in range(KS):
                    nc.tensor.matmul(pv, lhsT=xT[:, kk, ts(tt, 128)],
                                     rhs=wvT_sb[:, kk, :],
                                     start=False, stop=(kk == KS - 1),
                                     skip_group_check=True)
                nc.vector.tensor_copy(vh[:, tt, :, 0:64], pv)

        def attn(c, h4):
            """Attention for query chunk c (512 queries), head h4 (0..3)."""
            p, hh = divmod(h4, 2)
            hb = 64 * hh
            kTl, qTl = kT[p], qT[p]
            nbulk = 4 * c
            pa = {}   # bulk pair tiles: pa[jp] covers jt = 2jp, 2jp+1
            for jp in range(nbulk // 2):
                ps = psS.tile([128, 1024], f32, tag="s", name="s")
                for q2 in (0, 1):
                    jt = 2 * jp + q2
                    nc.tensor.matmul(ps[:, ts(q2, 512)],
                                     lhsT=kTl[hb:hb + 64, ts(jt, 128)],
                                     rhs=qTl[hb:hb + 64, ts(c, 512)],
                                     start=True, stop=True,
                                     skip_group_check=(q2 == 1))
                t_ = pbulk.tile([128, 1024], bf16, tag="pa", name="pa")
                nc.scalar.activation(t_, ps, EXP)
                pa[jp] = t_
            # band: subtile m covers j-tiles 4c..4c+m; pairs (0,3), (1,2)
            pb = {}
            for mpair in ((0, 3), (1, 2)):
                ps = psS.tile([128, 1024], f32, tag="s", name="s")
                off = 0
                offs = {}
                for m in mpair:
                    t_ = 4 * c + m
                    for mm in range(m):
                        nc.tensor.matmul(ps[:, off + 128 * mm:off + 128 * mm + 128],
                                         lhsT=kTl[hb:hb + 64, ts(4 * c + mm, 128)],
                                         rhs=qTl[hb:hb + 64, ts(t_, 128)],
                                         start=True, stop=True,
                                         skip_group_check=True)
                    dg = slice(off + 128 * m, off + 128 * m + 128)
                    nc.tensor.matmul(ps[:, dg], lhsT=negI_sb, rhs=Utri_sb,
                                     start=True, stop=False, skip_group_check=True)
                    nc.tensor.matmul(ps[:, dg],
                                     lhsT=kTl[hb:hb + 64, ts(t_, 128)],
                                     rhs=qTl[hb:hb + 64, ts(t_, 128)],
                                     start=False, stop=True,
                                     skip_group_check=True)
                    offs[m] = off
                    off += (m + 1) * 128
                pbt = pband.tile([128, 640], bf16, tag="pb", name="pb")
                nc.scalar.activation(pbt[:, 0:off], ps[:, 0:off], EXP)
                for m in mpair:
                    pb[m] = pbt[:, offs[m]:offs[m] + (m + 1) * 128]
            # window masked probabilities (Pool engine: SBUF-only)
            pw = {}
            for m in range(4):
                t_ = 4 * c + m
                t2 = ppw.tile([128, 256], bf16, tag="pw", name="pw")
                if t_ > 0:
                    if m == 0:
                        sub_src = pa[nbulk // 2 - 1][:, 512:640]
                    else:
                        sub_src = pb[m][:, ts(m - 1, 128)]
                    nc.gpsimd.tensor_tensor(t2[:, 0:128], sub_src,
                                            wm_sb[:, 0:128], MULT)
                nc.gpsimd.tensor_tensor(t2[:, 128:256], pb[m][:, ts(m, 128)],
                                        wm_sb[:, 128:256], MULT)
                pw[m] = t2
            # AV + normalization, two i-subtiles per psum tile
            o_chunk = state["oc"]
            for mh in (0, 1):
                av = psAV.tile([128, 260], f32, tag="av", name="av")
                for mq in (0, 1):
                    m = 2 * mh + mq
                    t_ = 4 * c + m
                    base = 130 * mq
                    OF = av[:, base:base + 65]
                    OW = av[:, base + 65:base + 130]
                    first = True
                    for jt in range(nbulk):
                        nc.tensor.matmul(
                            OF,
                            lhsT=pa[jt // 2][:, 512 * (jt % 2) + 128 * m:
                                             512 * (jt % 2) + 128 * m + 128],
                            rhs=vh[:, jt, h4, :],
                            start=first, stop=False, skip_group_check=True)
                        first = False
                    for mm in range(m + 1):
                        jt = 4 * c + mm
                        nc.tensor.matmul(OF, lhsT=pb[m][:, ts(mm, 128)],
                                         rhs=vh[:, jt, h4, :],
                                         start=first, stop=(mm == m),
                                         skip_group_check=True)
                        first = False
                    if t_ > 0:
                        nc.tensor.matmul(OW, lhsT=pw[m][:, 0:128],
                                         rhs=vh[:, t_ - 1, h4, :],
                                         start=True, stop=False,
                                         skip_group_check=True)
                    nc.tensor.matmul(OW, lhsT=pw[m][:, 128:256],
                                     rhs=vh[:, t_, h4, :],
                                     start=(t_ == 0), stop=True,
                                     skip_group_check=True)
                rcp = npool.tile([128, 4], f32, tag="rcp", name="rcp")
                nc.vector.reciprocal(rcp, av[:, 64::65])
                rsg = npool.tile([128, 2], f32, tag="rsg", name="rsg")
                nc.vector.tensor_scalar(rsg, rcp[:, 1::2], float(SG[0]), None, MULT)
                for mq in (0, 1):
                    m = 2 * mh + mq
                    base = 130 * mq
                    tf = npool.tile([128, 64], f32, tag="tf", name="tf")
                    nc.vector.tensor_scalar(tf, av[:, base:base + 64],
                                            rcp[:, 2 * mq:2 * mq + 1], None, MULT)
                    nc.vector.scalar_tensor_tensor(
                        o_chunk[:, m, 64 * h4:64 * h4 + 64],
                        av[:, base + 65:base + 129],
                        rsg[:, mq:mq + 1], tf, MULT, ADD)

        def transp_outproj(c):
            o_chunk = state["oc"]
            for dhalf in (0, 1):
                pst = psPO.tile([128, 512], f32, tag="po", name="po")
                for m in range(4):
                    nc.tensor.matmul(pst[:, ts(m, 128)],
                                     lhsT=o_chunk[:, m, ts(dhalf, 128)],
                                     rhs=idf_sb, is_transpose=True,
                                     start=True, stop=True,
                                     skip_group_check=True)
                nc.vector.tensor_copy(oT[:, dhalf, ts(c, 512)], pst)
            for et in range(8):
                ps = psPO.tile([128, 512], f32, tag="po", name="po")
                for kk in (0, 1):
                    nc.tensor.matmul(ps, lhsT=wo_sb[:, kk, ts(et, 128)],
                                     rhs=oT[:, kk, ts(c, 512)],
                                     start=(kk == 0), stop=(kk == 1))
                ob = opool.tile([128, 512], bf16, tag="ob", name="ob")
                if et % 2 == 0:
                    nc.vector.tensor_copy(ob, ps)
                else:
                    nc.scalar.activation(ob, ps, IDENT)
                nc.sync.dma_start(out=out[ts(et, 128), ts(c, 512)], in_=ob)

        # q/k for chunk 0 start as soon as their weights + x8 arrive
        proj_qk(0)
        # remaining (non-critical-path) const loads
        wvT_sb = cload(wvT, [128, KS, 256], bf16, "wvT")
        bvT_blk = consts.tile([128, 256], bf16, tag="bvT", name="bvT")
        nc.sync.dma_start(out=bvT_blk[0:1, :], in_=bvT)
        bvT_sb = bvT_blk[0:1, :]
        ones_blk = consts.tile([128, 128], bf16, tag="ones", name="ones")
        nc.vector.memset(ones_blk, 1.0)
        onesrow = ones_blk[0:1, :]
        wo_sb = cload(wo, [128, 2, DM], bf16, "wo")
        idf_sb = cload(idf, [128, 128], f32, "idf")
        proj_v(0)
        for c in range(NCH):
            state["oc"] = ochnk.tile([128, 4, 256], f32, tag="oc", name="oc")
            for h4 in range(4):
                attn(c, h4)
                if h4 == 1 and c + 1 < NCH:
                    proj_v(c + 1)
            if c + 1 < NCH:
                proj_qk(c + 1)
            transp_outproj(c)
        ctx.close()

    nc.compile()
    return nc


def _get_module():
    if "nc" not in _CACHE:
        _CACHE["nc"] = _build_module()
    return _CACHE["nc"]


def _prep_inputs(x, Wq, bq, Wk, bk, Wv, bv, Wo, bo, gate):
    x = np.asarray(x, np.float32)
    Wq = np.asarray(Wq, np.float32)
    Wk = np.asarray(Wk, np.float32)
    Wv = np.asarray(Wv, np.float32)
    Wo = np.asarray(Wo, np.float32)
    bq = np.asarray(bq, np.float32)
    bk = np.asarray(bk, np.float32)
    bv = np.asarray(bv, np.float32)
    SG[0] = float(1.0 / (1.0 + np.exp(-np.float32(gate))))

    fp8_qk = STAGE in ("B2",)
    qknp = FP8 if fp8_qk else BF16
    wscale = 8.0 if fp8_qk else 1.0                       # k weight scale
    qwscale = 8.0 if fp8_qk else 1.0 / np.sqrt(np.float32(DH))
    qbscale = 1.0 / np.sqrt(np.float32(DH))

    j = np.arange(128)[:, None]
    i = np.arange(128)[None, :]
    negI = (-30000.0 * (j == i)).astype(BF16)
    Utri = (j > i).astype(BF16)
    wm_sub = (j >= i + 65).astype(BF16)
    wm_diag = ((j <= i) & (j >= i - 63)).astype(BF16)
    msk = np.concatenate([negI, Utri, wm_sub, wm_diag], axis=1)
    idf = np.eye(128, dtype=np.float32)

    def wslice(Wmat, e0, scl, dt):
        Ws = (Wmat[e0:e0 + 128, :] * scl).astype(np.float32)  # [128 e', DM d]
        return np.ascontiguousarray(
            Ws.T.reshape(KS, 128, 128).transpose(1, 0, 2)).astype(dt)

    in_maps = []
    for core in range(NCORES):
        b, g = divmod(core, 4)
        e0 = g * 256
        xT_full = np.ascontiguousarray(x[b].T)            # [DM, T]
        xTc = np.ascontiguousarray(
            xT_full.reshape(KS, 128, NCH, 512).transpose(2, 1, 0, 3))
        wq_c = np.stack([wslice(Wq, e0 + 128 * p, qwscale, qknp) for p in (0, 1)])
        wk_c = np.stack([wslice(Wk, e0 + 128 * p, wscale, qknp) for p in (0, 1)])
        Wvs = Wv[e0:e0 + 256, :].astype(np.float32)       # [256 e', DM]
        wvT_c = np.ascontiguousarray(
            Wvs.T.reshape(KS, 128, 256).transpose(1, 0, 2)).astype(BF16)
        Wos = Wo[:, e0:e0 + 256]                          # [DM e, 256 d]
        wo_c = np.ascontiguousarray(
            Wos.T.reshape(2, 128, DM).transpose(1, 0, 2)).astype(BF16)
        bqk_c = np.stack([bq[e0:e0 + 128] * qbscale,
                          bq[e0 + 128:e0 + 256] * qbscale,
                          bk[e0:e0 + 128], bk[e0 + 128:e0 + 256]],
                         axis=1).astype(np.float32)       # [128, 4]
        in_maps.append({
            "xTd": xTc.astype(BF16), "x8d": xTc.astype(qknp),
            "wqk": np.ascontiguousarray(np.stack([wq_c, wk_c]).transpose(2, 0, 1, 3, 4)), "wvT": wvT_c, "wo": wo_c,
            "bqk": bqk_c,
            "bvT": bv[e0:e0 + 256].reshape(1, 256).astype(BF16),
            "msk": msk, "idf": idf,
        })
    return in_maps


def _run(nc, in_maps, **kw):
    from concourse.bass_utils import run_bass_kernel_spmd
    from concourse.bass_interp import get_hw_module
    old = nc.m
    nc.m = get_hw_module(nc.m)
    try:
        res = run_bass_kernel_spmd(nc, in_maps, core_ids=list(range(NCORES)), **kw)
    finally:
        nc.m = old
    return res


def kernel(x, Wq, bq, Wk, bk, Wv, bv, Wo, bo, gate):
    in_maps = _prep_inputs(x, Wq, bq, Wk, bk, Wv, bv, Wo, bo, gate)
    nc = _get_module()
    res = _run(nc, in_maps)
    bo = np.asarray(bo, np.float32)
    out = np.zeros((B, T, DM), np.float32)
    for core in range(NCORES):
        b = core // 4
        out[b] += res.results[core]["out"].astype(np.float32).T
    out += bo[None, None, :]
    return out


# revision 4
# speedup vs baseline: 1.0261x; 1.0023x over previous
"""EvoMultiheadSelfAttention Trainium2 kernel (8 NeuronCores, SPMD), v3.

Sharding: core = (batch b, group of 4 heads). Per core: project q/k/v for
its 4 heads, causal full attention + 64-wide sliding-window attention,
combine with sigmoid(gate), partial output projection over its 256-dim
d-slice. Host sums 4 partials per batch and adds bo.

Key structure:
  - Scores transposed sT[j, i] (keys on partitions); AV produces O[i, d]
    (lhsT = p tiles, rhs = v natural [t, d] + ones lane -> per-partition
    softmax denominators; normalization via per-partition tensor_scalar).
  - v projected DIRECTLY into [t, d] layout (lhsT = xT): no transposes.
  - q/k projections in fp8e4m3 + DoubleRow (2x PE throughput); v path
    stays bf16 (v quantization error dominates otherwise).
  - Causal mask on diag tiles via an extra PE matmul writing -30000 into
    PSUM before the score matmul (group kept contiguous - an intervening
    start=True on a bank wipes open accumulation groups).
  - Bulk score pairs in one 2-bank PSUM tile ([128,1024] exp ops); band
    tiles paired (0,3)/(1,2) into [128,640] exp ops.
  - Window masks multiplied on the Pool engine (SBUF-only, otherwise idle).
  - O transposed to oT[d, t] by PE transpose (f32); outproj per chunk;
    psum->sbuf out copies split DVE/Act; stores via SP DMA.
"""

import os
import numpy as np
import ml_dtypes

B, T, DM, H, WIN = 2, 2048, 1024, 16, 64
DH = DM // H          # 64
NCORES = 8
KS = DM // 128        # 8 d-subtiles
NT = T // 128         # 16 j/i tiles
NCH = T // 512        # 4 chunks of 512
BF16 = ml_dtypes.bfloat16
FP8 = ml_dtypes.float8_e4m3

STAGE = os.environ.get("EVO_STAGE", "B2")

_CACHE: dict = {}
SG = [0.11920292202211755]  # sigmoid(gate); set by _prep_inputs before build


def _build_module():
    import contextlib
    import concourse.bass as bass  # noqa: F401
    import concourse.mybir as mybir
    import concourse.tile as tile
    from concourse import bacc
    from concourse.bass import ts

    f32 = mybir.dt.float32
    bf16 = mybir.dt.bfloat16
    fp8 = mybir.dt.float8e4
    EXP = mybir.ActivationFunctionType.Exp
    IDENT = mybir.ActivationFunctionType.Identity
    MULT = mybir.AluOpType.mult
    ADD = mybir.AluOpType.add
    DR = mybir.MatmulPerfMode.DoubleRow

    fp8_qk = STAGE in ("B2",)
    qkdt = fp8 if fp8_qk else bf16
    qcs = (1.0 / 64.0) if fp8_qk else 1.0   # q copy scale (descale + 1/sqrt(dh))
    kcs = (1.0 / 8.0) if fp8_qk else 1.0    # k copy scale

    nc = bacc.Bacc("TRN2", target_bir_lowering=False, debug=False, num_devices=NCORES)

    def din(name, shape, dt):
        return nc.dram_tensor(name, shape, dt, kind="ExternalInput").ap()

    xTd = din("xTd", [NCH, 128, KS, 512], bf16)    # x[b]^T chunked (bf16, v path)
    x8d = din("x8d", [NCH, 128, KS, 512], qkdt)    # fp8 copy for q/k path
    wqk = din("wqk", [128, 2, 2, KS, 128], qkdt)   # [dp, q/k, p, ks, e']
    wvT = din("wvT", [128, KS, 256], bf16)
    wo = din("wo", [128, 2, DM], bf16)
    bqk = din("bqk", [128, 4], f32)                # [bq p0, bq p1, bk p0, bk p1]
    bvT = din("bvT", [1, 256], bf16)
    # masks packed: [negI | Utri | wm_sub | wm_diag]
    msk = din("msk", [128, 512], bf16)
    idf = din("idf", [128, 128], bf16)             # identity for PE transpose
    out = nc.dram_tensor("out", [DM, T], bf16, kind="ExternalOutput").ap()

    with tile.TileContext(nc) as tc:
        ctx = contextlib.ExitStack()
        consts = ctx.enter_context(tc.tile_pool(name="consts", bufs=1))
        big = ctx.enter_context(tc.tile_pool(name="big", bufs=1))
        pbulk = ctx.enter_context(tc.tile_pool(name="pbulk", bufs=14))
        pband = ctx.enter_context(tc.tile_pool(name="pband", bufs=6))
        ppw = ctx.enter_context(tc.tile_pool(name="ppw", bufs=9))
        npool = ctx.enter_context(tc.tile_pool(name="npool", bufs=4))
        ochnk = ctx.enter_context(tc.tile_pool(name="ochnk", bufs=2))
        opool = ctx.enter_context(tc.tile_pool(name="opool", bufs=6))
        psS = ctx.enter_context(tc.tile_pool(name="psS", bufs=2, space="PSUM"))
        psAV = ctx.enter_context(tc.tile_pool(name="psAV", bufs=2, space="PSUM"))
        psPO = ctx.enter_context(tc.tile_pool(name="psPO", bufs=2, space="PSUM"))

        def cload(ap_in, shape, dt, tag):
            t_ = consts.tile(shape, dt, tag=tag, name=tag)
            nc.sync.dma_start(out=t_, in_=ap_in)
            return t_

        # critical-path loads first
        wqk_sb = cload(wqk, [128, 2, 2, KS, 128], qkdt, "wqk")
        bqk_sb = cload(bqk, [128, 4], f32, "bqk")
        msk_sb = cload(msk, [128, 512], bf16, "msk")
        negI_sb = msk_sb[:, 0:128]
        Utri_sb = msk_sb[:, 128:256]
        wm_sb = msk_sb[:, 256:512]

        warm = consts.tile([128, 1], f32, tag="warm", name="warm")
        nc.vector.memset(warm, 0.0)
        warm2 = consts.tile([128, 1], bf16, tag="warm2", name="warm2")
        nc.scalar.activation(warm2, warm, EXP)

        # xT [dp, ks, t]
        xT = big.tile([128, KS, T], bf16, tag="xT", name="xT")
        x8 = (big.tile([128, KS, T], qkdt, tag="x8", name="x8")
              if fp8_qk else xT)
        qT = [big.tile([128, T], bf16, tag=f"qT{p}", name=f"qT{p}") for p in (0, 1)]
        kT = [big.tile([128, T], bf16, tag=f"kT{p}", name=f"kT{p}") for p in (0, 1)]
        vh = big.tile([128, NT, 4, 65], bf16, tag="vh", name="vh")
        nc.vector.memset(vh, 1.0)
        oT = big.tile([128, 2, T], bf16, tag="oT", name="oT")

        state = {"oc": None}

        def proj_qk(c4):
            if fp8_qk:
                nc.sync.dma_start(out=x8[:, :, ts(c4, 512)], in_=x8d[c4])
            for p in (0, 1):
                for qk, dst, cs in ((0, qT[p], qcs), (1, kT[p], kcs)):
                    w_sb = wqk_sb[:, qk, p]
                    b_sb = bqk_sb[:, 2 * qk + p:2 * qk + p + 1]
                    ps = psPO.tile([128, 512], f32, tag="po", name="po")
                    if fp8_qk:
                        for kk in range(4):
                            nc.tensor.matmul(ps, lhsT=w_sb[:, 2 * kk:2 * kk + 2, :],
                                             rhs=x8[:, 2 * kk:2 * kk + 2, ts(c4, 512)],
                                             perf_mode=DR,
                                             start=(kk == 0), stop=(kk == 3))
                    else:
                        for kk in range(KS):
                            nc.tensor.matmul(ps, lhsT=w_sb[:, kk, :],
                                             rhs=x8[:, kk, ts(c4, 512)],
                                             start=(kk == 0), stop=(kk == KS - 1))
                    nc.vector.tensor_scalar(dst[:, ts(c4, 512)], ps, cs, b_sb,
                                            MULT, ADD)

        def proj_v(c4):
            nc.sync.dma_start(out=xT[:, :, ts(c4, 512)], in_=xTd[c4])
            for tt in range(4 * c4, 4 * c4 + 4):
                ps = psPO.tile([128, 512], f32, tag="po", name="po")
                pv = ps[:, 0:256]
                nc.tensor.matmul(pv, lhsT=onesrow, rhs=bvT_sb,
                                 start=True, stop=False, skip_group_check=True)
                for kk in range(KS):
                    nc.tensor.matmul(pv, lhsT=xT[:, kk, ts(tt, 128)],
                                     rhs=wvT_sb[:, kk, :],
                                     start=False, stop=(kk == KS - 1),
                                     skip_group_check=True)
                nc.vector.tensor_copy(vh[:, tt, :, 0:64], pv)

        def attn(c, h4):
            """Attention for query chunk c (512 queries), head h4 (0..3)."""
            p, hh = divmod(h4, 2)
            hb = 64 * hh
            kTl, qTl = kT[p], qT[p]
            nbulk = 4 * c
            pa = {}   # bulk pair tiles: pa[jp] covers jt = 2jp, 2jp+1
            for jp in range(nbulk // 2):
                ps = psS.tile([128, 1024], f32, tag="s", name="s")
                for q2 in (0, 1):
                    jt = 2 * jp + q2
                    nc.tensor.matmul(ps[:, ts(q2, 512)],
                                     lhsT=kTl[hb:hb + 64, ts(jt, 128)],
                                     rhs=qTl[hb:hb + 64, ts(c, 512)],
                                     start=True, stop=True,
                                     skip_group_check=(q2 == 1))
                t_ = pbulk.tile([128, 1024], bf16, tag="pa", name="pa")
                nc.scalar.activation(t_, ps, EXP)
                pa[jp] = t_
            # band: subtile m covers j-tiles 4c..4c+m; pairs (0,3), (1,2)
            pb = {}
            for mpair in ((0, 3), (1, 2)):
                ps = psS.tile([128, 1024], f32, tag="s", name="s")
                off = 0
                offs = {}
                for m in mpair:
                    t_ = 4 * c + m
                    for mm in range(m):
                        nc.tensor.matmul(ps[:, off + 128 * mm:off + 128 * mm + 128],
                                         lhsT=kTl[hb:hb + 64, ts(4 * c + mm, 128)],
                                         rhs=qTl[hb:hb + 64, ts(t_, 128)],
                                         start=True, stop=True,
                                         skip_group_check=True)
                    dg = slice(off + 128 * m, off + 128 * m + 128)
                    nc.tensor.matmul(ps[:, dg], lhsT=negI_sb, rhs=Utri_sb,
                                     start=True, stop=False, skip_group_check=True)
                    nc.tensor.matmul(ps[:, dg],
                                     lhsT=kTl[hb:hb + 64, ts(t_, 128)],
                                     rhs=qTl[hb:hb + 64, ts(t_, 128)],
                                     start=False, stop=True,
                                     skip_group_check=True)
                    offs[m] = off
                    off += (m + 1) * 128
                pbt = pband.tile([128, 640], bf16, tag="pb", name="pb")
                nc.scalar.activation(pbt[:, 0:off], ps[:, 0:off], EXP)
                for m in mpair:
                    pb[m] = pbt[:, offs[m]:offs[m] + (m + 1) * 128]
            # window masked probabilities (Pool engine: SBUF-only)
            pw = {}
            for m in range(4):
                t_ = 4 * c + m
                t2 = ppw.tile([128, 256], bf16, tag="pw", name="pw")
                if t_ > 0:
                    if m == 0:
                        sub_src = pa[nbulk // 2 - 1][:, 512:640]
                    else:
                        sub_src = pb[m][:, ts(m - 1, 128)]
                    nc.gpsimd.tensor_tensor(t2[:, 0:128], sub_src,
                                            wm_sb[:, 0:128], MULT)
                nc.vector.tensor_tensor(t2[:, 128:256], pb[m][:, ts(m, 128)],
                                        wm_sb[:, 128:256], MULT)
                pw[m] = t2
            # AV + normalization, two i-subtiles per psum tile
            o_chunk = state["oc"]
            for mh in (0, 1):
                av = psAV.tile([128, 260], f32, tag="av", name="av")
                for mq in (0, 1):
                    m = 2 * mh + mq
                    t_ = 4 * c + m
                    base = 130 * mq
                    OF = av[:, base:base + 65]
                    OW = av[:, base + 65:base + 130]
                    first = True
                    for jt in range(nbulk):
                        nc.tensor.matmul(
                            OF,
                            lhsT=pa[jt // 2][:, 512 * (jt % 2) + 128 * m:
                                             512 * (jt % 2) + 128 * m + 128],
                            rhs=vh[:, jt, h4, :],
                            start=first, stop=False, skip_group_check=True)
                        first = False
                    for mm in range(m + 1):
                        jt = 4 * c + mm
                        nc.tensor.matmul(OF, lhsT=pb[m][:, ts(mm, 128)],
                                         rhs=vh[:, jt, h4, :],
                                         start=first, stop=(mm == m),
                                         skip_group_check=True)
                        first = False
                    if t_ > 0:
                        nc.tensor.matmul(OW, lhsT=pw[m][:, 0:128],
                                         rhs=vh[:, t_ - 1, h4, :],
                                         start=True, stop=False,
                                         skip_group_check=True)
                    nc.tensor.matmul(OW, lhsT=pw[m][:, 128:256],
                                     rhs=vh[:, t_, h4, :],
                                     start=(t_ == 0), stop=True,
                                     skip_group_check=True)
                rcp = npool.tile([128, 4], f32, tag="rcp", name="rcp")
                nc.vector.reciprocal(rcp, av[:, 64::65])
                rsg = npool.tile([128, 2], f32, tag="rsg", name="rsg")
                nc.vector.tensor_scalar(rsg, rcp[:, 1::2], float(SG[0]), None, MULT)
                for mq in (0, 1):
                    m = 2 * mh + mq
                    base = 130 * mq
                    tf = npool.tile([128, 64], f32, tag="tf", name="tf")
                    nc.vector.tensor_scalar(tf, av[:, base:base + 64],
                                            rcp[:, 2 * mq:2 * mq + 1], None, MULT)
                    nc.vector.scalar_tensor_tensor(
                        o_chunk[:, m, 64 * h4:64 * h4 + 64],
                        av[:, base + 65:base + 129],
                        rsg[:, mq:mq + 1], tf, MULT, ADD)

        def transp_outproj(c):
            o_chunk = state["oc"]
            for dhalf in (0, 1):
                pst = psPO.tile([128, 512], bf16, tag="po", name="po",
                                  padded_shape=[128, 1024])
                for m in range(4):
                    nc.tensor.matmul(pst[:, ts(m, 128)],
                                     lhsT=o_chunk[:, m, ts(dhalf, 128)],
                                     rhs=idf_sb, is_transpose=True,
                                     start=True, stop=True,
                                     skip_group_check=True)
                nc.vector.tensor_copy(oT[:, dhalf, ts(c, 512)], pst)
            for et in range(8):
                ps = psPO.tile([128, 512], f32, tag="po", name="po")
                for kk in (0, 1):
                    nc.tensor.matmul(ps, lhsT=wo_sb[:, kk, ts(et, 128)],
                                     rhs=oT[:, kk, ts(c, 512)],
                                     start=(kk == 0), stop=(kk == 1))
                ob = opool.tile([128, 512], bf16, tag="ob", name="ob")
                if et % 2 == 0:
                    nc.vector.tensor_copy(ob, ps)
                else:
                    nc.scalar.activation(ob, ps, IDENT)
                nc.sync.dma_start(out=out[ts(et, 128), ts(c, 512)], in_=ob)

        # q/k for chunk 0 start as soon as their weights + x8 arrive
        proj_qk(0)
        # remaining (non-critical-path) const loads
        wvT_sb = cload(wvT, [128, KS, 256], bf16, "wvT")
        bvT_blk = consts.tile([128, 256], bf16, tag="bvT", name="bvT")
        nc.sync.dma_start(out=bvT_blk[0:1, :], in_=bvT)
        bvT_sb = bvT_blk[0:1, :]
        ones_blk = consts.tile([128, 128], bf16, tag="ones", name="ones")
        nc.vector.memset(ones_blk, 1.0)
        onesrow = ones_blk[0:1, :]
        wo_sb = cload(wo, [128, 2, DM], bf16, "wo")
        idf_sb = cload(idf, [128, 128], bf16, "idf")
        proj_v(0)
        for c in range(NCH):
            state["oc"] = ochnk.tile([128, 4, 256], bf16, tag="oc", name="oc")
            for h4 in range(4):
                attn(c, h4)
                if h4 == 1 and c + 1 < NCH:
                    proj_v(c + 1)
            if c + 1 < NCH:
                proj_qk(c + 1)
            transp_outproj(c)
        ctx.close()

    nc.compile()
    return nc


def _get_module():
    if "nc" not in _CACHE:
        _CACHE["nc"] = _build_module()
    return _CACHE["nc"]


def _prep_inputs(x, Wq, bq, Wk, bk, Wv, bv, Wo, bo, gate):
    x = np.asarray(x, np.float32)
    Wq = np.asarray(Wq, np.float32)
    Wk = np.asarray(Wk, np.float32)
    Wv = np.asarray(Wv, np.float32)
    Wo = np.asarray(Wo, np.float32)
    bq = np.asarray(bq, np.float32)
    bk = np.asarray(bk, np.float32)
    bv = np.asarray(bv, np.float32)
    SG[0] = float(1.0 / (1.0 + np.exp(-np.float32(gate))))

    fp8_qk = STAGE in ("B2",)
    qknp = FP8 if fp8_qk else BF16
    wscale = 8.0 if fp8_qk else 1.0                       # k weight scale
    qwscale = 8.0 if fp8_qk else 1.0 / np.sqrt(np.float32(DH))
    qbscale = 1.0 / np.sqrt(np.float32(DH))

    j = np.arange(128)[:, None]
    i = np.arange(128)[None, :]
    negI = (-30000.0 * (j == i)).astype(BF16)
    Utri = (j > i).astype(BF16)
    wm_sub = (j >= i + 65).astype(BF16)
    wm_diag = ((j <= i) & (j >= i - 63)).astype(BF16)
    msk = np.concatenate([negI, Utri, wm_sub, wm_diag], axis=1)
    idf = np.eye(128).astype(BF16)

    def wslice(Wmat, e0, scl, dt):
        Ws = (Wmat[e0:e0 + 128, :] * scl).astype(np.float32)  # [128 e', DM d]
        return np.ascontiguousarray(
            Ws.T.reshape(KS, 128, 128).transpose(1, 0, 2)).astype(dt)

    in_maps = []
    for core in range(NCORES):
        b, g = divmod(core, 4)
        e0 = g * 256
        xT_full = np.ascontiguousarray(x[b].T)            # [DM, T]
        xTc = np.ascontiguousarray(
            xT_full.reshape(KS, 128, NCH, 512).transpose(2, 1, 0, 3))
        wq_c = np.stack([wslice(Wq, e0 + 128 * p, qwscale, qknp) for p in (0, 1)])
        wk_c = np.stack([wslice(Wk, e0 + 128 * p, wscale, qknp) for p in (0, 1)])
        Wvs = Wv[e0:e0 + 256, :].astype(np.float32)       # [256 e', DM]
        wvT_c = np.ascontiguousarray(
            Wvs.T.reshape(KS, 128, 256).transpose(1, 0, 2)).astype(BF16)
        Wos = Wo[:, e0:e0 + 256]                          # [DM e, 256 d]
        wo_c = np.ascontiguousarray(
            Wos.T.reshape(2, 128, DM).transpose(1, 0, 2)).astype(BF16)
        bqk_c = np.stack([bq[e0:e0 + 128] * qbscale,
                          bq[e0 + 128:e0 + 256] * qbscale,
                          bk[e0:e0 + 128], bk[e0 + 128:e0 + 256]],
                         axis=1).astype(np.float32)       # [128, 4]
        in_maps.append({
            "xTd": xTc.astype(BF16), "x8d": xTc.astype(qknp),
            "wqk": np.ascontiguousarray(np.stack([wq_c, wk_c]).transpose(2, 0, 1, 3, 4)), "wvT": wvT_c, "wo": wo_c,
            "bqk": bqk_c,
            "bvT": bv[e0:e0 + 256].reshape(1, 256).astype(BF16),
            "msk": msk, "idf": idf,
        })
    return in_maps


def _run(nc, in_maps, **kw):
    from concourse.bass_utils import run_bass_kernel_spmd
    from concourse.bass_interp import get_hw_module
    old = nc.m
    nc.m = get_hw_module(nc.m)
    try:
        res = run_bass_kernel_spmd(nc, in_maps, core_ids=list(range(NCORES)), **kw)
    finally:
        nc.m = old
    return res


def kernel(x, Wq, bq, Wk, bk, Wv, bv, Wo, bo, gate):
    in_maps = _prep_inputs(x, Wq, bq, Wk, bk, Wv, bv, Wo, bo, gate)
    nc = _get_module()
    res = _run(nc, in_maps)
    bo = np.asarray(bo, np.float32)
    out = np.zeros((B, T, DM), np.float32)
    for core in range(NCORES):
        b = core // 4
        out[b] += res.results[core]["out"].astype(np.float32).T
    out += bo[None, None, :]
    return out


# revision 5
# speedup vs baseline: 1.0274x; 1.0013x over previous
"""EvoMultiheadSelfAttention Trainium2 kernel (8 NeuronCores, SPMD), v3.

Sharding: core = (batch b, group of 4 heads). Per core: project q/k/v for
its 4 heads, causal full attention + 64-wide sliding-window attention,
combine with sigmoid(gate), partial output projection over its 256-dim
d-slice. Host sums 4 partials per batch and adds bo.

Key structure:
  - Scores transposed sT[j, i] (keys on partitions); AV produces O[i, d]
    (lhsT = p tiles, rhs = v natural [t, d] + ones lane -> per-partition
    softmax denominators; normalization via per-partition tensor_scalar).
  - v projected DIRECTLY into [t, d] layout (lhsT = xT): no transposes.
  - q/k projections in fp8e4m3 + DoubleRow (2x PE throughput); v path
    stays bf16 (v quantization error dominates otherwise).
  - Causal mask on diag tiles via an extra PE matmul writing -30000 into
    PSUM before the score matmul (group kept contiguous - an intervening
    start=True on a bank wipes open accumulation groups).
  - Bulk score pairs in one 2-bank PSUM tile ([128,1024] exp ops); band
    tiles paired (0,3)/(1,2) into [128,640] exp ops.
  - Window masks multiplied on the Pool engine (SBUF-only, otherwise idle).
  - O transposed to oT[d, t] by PE transpose (f32); outproj per chunk;
    psum->sbuf out copies split DVE/Act; stores via SP DMA.
"""

import os
import numpy as np
import ml_dtypes

B, T, DM, H, WIN = 2, 2048, 1024, 16, 64
DH = DM // H          # 64
NCORES = 8
KS = DM // 128        # 8 d-subtiles
NT = T // 128         # 16 j/i tiles
NCH = T // 512        # 4 chunks of 512
BF16 = ml_dtypes.bfloat16
FP8 = ml_dtypes.float8_e4m3

STAGE = os.environ.get("EVO_STAGE", "B2")

_CACHE: dict = {}
SG = [0.11920292202211755]  # sigmoid(gate); set by _prep_inputs before build


def _build_module():
    import contextlib
    import concourse.bass as bass  # noqa: F401
    import concourse.mybir as mybir
    import concourse.tile as tile
    from concourse import bacc
    from concourse.bass import ts

    f32 = mybir.dt.float32
    bf16 = mybir.dt.bfloat16
    fp8 = mybir.dt.float8e4
    EXP = mybir.ActivationFunctionType.Exp
    IDENT = mybir.ActivationFunctionType.Identity
    MULT = mybir.AluOpType.mult
    ADD = mybir.AluOpType.add
    DR = mybir.MatmulPerfMode.DoubleRow

    fp8_qk = STAGE in ("B2",)
    qkdt = fp8 if fp8_qk else bf16
    qcs = (1.0 / 64.0) if fp8_qk else 1.0   # q copy scale (descale + 1/sqrt(dh))
    kcs = (1.0 / 8.0) if fp8_qk else 1.0    # k copy scale

    nc = bacc.Bacc("TRN2", target_bir_lowering=False, debug=False, num_devices=NCORES)

    def din(name, shape, dt):
        return nc.dram_tensor(name, shape, dt, kind="ExternalInput").ap()

    xTd = din("xTd", [NCH, 128, KS, 512], bf16)    # x[b]^T chunked (bf16, v path)
    x8d = din("x8d", [NCH, 128, KS, 512], qkdt)    # fp8 copy for q/k path
    wqk = din("wqk", [128, 2, 2, KS, 128], qkdt)   # [dp, q/k, p, ks, e']
    wvT = din("wvT", [128, KS, 256], bf16)
    wo = din("wo", [128, 2, DM], bf16)
    bqk = din("bqk", [128, 4], f32)                # [bq p0, bq p1, bk p0, bk p1]
    bvT = din("bvT", [1, 256], bf16)
    # masks packed: [negI | Utri | wm_sub | wm_diag]
    msk = din("msk", [128, 512], bf16)
    idf = din("idf", [128, 128], bf16)             # identity for PE transpose
    out = nc.dram_tensor("out", [DM, T], bf16, kind="ExternalOutput").ap()

    with tile.TileContext(nc) as tc:
        ctx = contextlib.ExitStack()
        consts = ctx.enter_context(tc.tile_pool(name="consts", bufs=1))
        big = ctx.enter_context(tc.tile_pool(name="big", bufs=1))
        pbulk = ctx.enter_context(tc.tile_pool(name="pbulk", bufs=14))
        pband = ctx.enter_context(tc.tile_pool(name="pband", bufs=6))
        ppw = ctx.enter_context(tc.tile_pool(name="ppw", bufs=9))
        npool = ctx.enter_context(tc.tile_pool(name="npool", bufs=4))
        ochnk = ctx.enter_context(tc.tile_pool(name="ochnk", bufs=2))
        opool = ctx.enter_context(tc.tile_pool(name="opool", bufs=6))
        psS = ctx.enter_context(tc.tile_pool(name="psS", bufs=2, space="PSUM"))
        psAV = ctx.enter_context(tc.tile_pool(name="psAV", bufs=2, space="PSUM"))
        psPO = ctx.enter_context(tc.tile_pool(name="psPO", bufs=2, space="PSUM"))

        def cload(ap_in, shape, dt, tag):
            t_ = consts.tile(shape, dt, tag=tag, name=tag)
            nc.sync.dma_start(out=t_, in_=ap_in)
            return t_

        # critical-path loads first: q weights, biases, then x8 chunk 0
        # arrives before k weights / masks (HWDGE+DMA serialize transfers)
        wqk_sb = consts.tile([128, 2, 2, KS, 128], qkdt, tag="wqk", name="wqk")
        nc.sync.dma_start(out=wqk_sb[:, 0], in_=wqk[:, 0])
        bqk_sb = cload(bqk, [128, 4], f32, "bqk")

        warm = consts.tile([128, 1], f32, tag="warm", name="warm")
        nc.vector.memset(warm, 0.0)
        warm2 = consts.tile([128, 1], bf16, tag="warm2", name="warm2")
        nc.scalar.activation(warm2, warm, EXP)

        # xT [dp, ks, t]
        xT = big.tile([128, KS, T], bf16, tag="xT", name="xT")
        x8 = (big.tile([128, KS, T], qkdt, tag="x8", name="x8")
              if fp8_qk else xT)
        qT = [big.tile([128, T], bf16, tag=f"qT{p}", name=f"qT{p}") for p in (0, 1)]
        kT = [big.tile([128, T], bf16, tag=f"kT{p}", name=f"kT{p}") for p in (0, 1)]
        vh = big.tile([128, NT, 4, 65], bf16, tag="vh", name="vh")
        nc.vector.memset(vh, 1.0)
        oT = big.tile([128, 2, T], bf16, tag="oT", name="oT")

        state = {"oc": None}

        def proj_qk(c4):
            if fp8_qk:
                nc.sync.dma_start(out=x8[:, :, ts(c4, 512)], in_=x8d[c4])
            if c4 == 0:
                nc.sync.dma_start(out=wqk_sb[:, 1], in_=wqk[:, 1])
            for p in (0, 1):
                for qk, dst, cs in ((0, qT[p], qcs), (1, kT[p], kcs)):
                    w_sb = wqk_sb[:, qk, p]
                    b_sb = bqk_sb[:, 2 * qk + p:2 * qk + p + 1]
                    ps = psPO.tile([128, 512], f32, tag="po", name="po")
                    if fp8_qk:
                        for kk in range(4):
                            nc.tensor.matmul(ps, lhsT=w_sb[:, 2 * kk:2 * kk + 2, :],
                                             rhs=x8[:, 2 * kk:2 * kk + 2, ts(c4, 512)],
                                             perf_mode=DR,
                                             start=(kk == 0), stop=(kk == 3))
                    else:
                        for kk in range(KS):
                            nc.tensor.matmul(ps, lhsT=w_sb[:, kk, :],
                                             rhs=x8[:, kk, ts(c4, 512)],
                                             start=(kk == 0), stop=(kk == KS - 1))
                    nc.vector.tensor_scalar(dst[:, ts(c4, 512)], ps, cs, b_sb,
                                            MULT, ADD)

        def proj_v(c4):
            nc.sync.dma_start(out=xT[:, :, ts(c4, 512)], in_=xTd[c4])
            for tt in range(4 * c4, 4 * c4 + 4):
                ps = psPO.tile([128, 512], f32, tag="po", name="po")
                pv = ps[:, 0:256]
                nc.tensor.matmul(pv, lhsT=onesrow, rhs=bvT_sb,
                                 start=True, stop=False, skip_group_check=True)
                for kk in range(KS):
                    nc.tensor.matmul(pv, lhsT=xT[:, kk, ts(tt, 128)],
                                     rhs=wvT_sb[:, kk, :],
                                     start=False, stop=(kk == KS - 1),
                                     skip_group_check=True)
                nc.vector.tensor_copy(vh[:, tt, :, 0:64], pv)

        def attn(c, h4):
            """Attention for query chunk c (512 queries), head h4 (0..3)."""
            p, hh = divmod(h4, 2)
            hb = 64 * hh
            kTl, qTl = kT[p], qT[p]
            nbulk = 4 * c
            pa = {}   # bulk pair tiles: pa[jp] covers jt = 2jp, 2jp+1
            for jp in range(nbulk // 2):
                ps = psS.tile([128, 1024], f32, tag="s", name="s")
                for q2 in (0, 1):
                    jt = 2 * jp + q2
                    nc.tensor.matmul(ps[:, ts(q2, 512)],
                                     lhsT=kTl[hb:hb + 64, ts(jt, 128)],
                                     rhs=qTl[hb:hb + 64, ts(c, 512)],
                                     start=True, stop=True,
                                     skip_group_check=(q2 == 1))
                t_ = pbulk.tile([128, 1024], bf16, tag="pa", name="pa")
                nc.scalar.activation(t_, ps, EXP)
                pa[jp] = t_
            # band: subtile m covers j-tiles 4c..4c+m; pairs (0,3), (1,2)
            pb = {}
            for mpair in ((0, 3), (1, 2)):
                ps = psS.tile([128, 1024], f32, tag="s", name="s")
                off = 0
                offs = {}
                for m in mpair:
                    t_ = 4 * c + m
                    for mm in range(m):
                        nc.tensor.matmul(ps[:, off + 128 * mm:off + 128 * mm + 128],
                                         lhsT=kTl[hb:hb + 64, ts(4 * c + mm, 128)],
                                         rhs=qTl[hb:hb + 64, ts(t_, 128)],
                                         start=True, stop=True,
                                         skip_group_check=True)
                    dg = slice(off + 128 * m, off + 128 * m + 128)
                    nc.tensor.matmul(ps[:, dg], lhsT=negI_sb, rhs=Utri_sb,
                                     start=True, stop=False, skip_group_check=True)
                    nc.tensor.matmul(ps[:, dg],
                                     lhsT=kTl[hb:hb + 64, ts(t_, 128)],
                                     rhs=qTl[hb:hb + 64, ts(t_, 128)],
                                     start=False, stop=True,
                                     skip_group_check=True)
                    offs[m] = off
                    off += (m + 1) * 128
                pbt = pband.tile([128, 640], bf16, tag="pb", name="pb")
                nc.scalar.activation(pbt[:, 0:off], ps[:, 0:off], EXP)
                for m in mpair:
                    pb[m] = pbt[:, offs[m]:offs[m] + (m + 1) * 128]
            # window masked probabilities (Pool engine: SBUF-only)
            pw = {}
            for m in range(4):
                t_ = 4 * c + m
                t2 = ppw.tile([128, 256], bf16, tag="pw", name="pw")
                if t_ > 0:
                    if m == 0:
                        sub_src = pa[nbulk // 2 - 1][:, 512:640]
                    else:
                        sub_src = pb[m][:, ts(m - 1, 128)]
                    nc.gpsimd.tensor_tensor(t2[:, 0:128], sub_src,
                                            wm_sb[:, 0:128], MULT)
                nc.vector.tensor_tensor(t2[:, 128:256], pb[m][:, ts(m, 128)],
                                        wm_sb[:, 128:256], MULT)
                pw[m] = t2
            # AV + normalization, two i-subtiles per psum tile
            o_chunk = state["oc"]
            for mh in (0, 1):
                av = psAV.tile([128, 260], f32, tag="av", name="av")
                for mq in (0, 1):
                    m = 2 * mh + mq
                    t_ = 4 * c + m
                    base = 130 * mq
                    OF = av[:, base:base + 65]
                    OW = av[:, base + 65:base + 130]
                    first = True
                    for jt in range(nbulk):
                        nc.tensor.matmul(
                            OF,
                            lhsT=pa[jt // 2][:, 512 * (jt % 2) + 128 * m:
                                             512 * (jt % 2) + 128 * m + 128],
                            rhs=vh[:, jt, h4, :],
                            start=first, stop=False, skip_group_check=True)
                        first = False
                    for mm in range(m + 1):
                        jt = 4 * c + mm
                        nc.tensor.matmul(OF, lhsT=pb[m][:, ts(mm, 128)],
                                         rhs=vh[:, jt, h4, :],
                                         start=first, stop=(mm == m),
                                         skip_group_check=True)
                        first = False
                    if t_ > 0:
                        nc.tensor.matmul(OW, lhsT=pw[m][:, 0:128],
                                         rhs=vh[:, t_ - 1, h4, :],
                                         start=True, stop=False,
                                         skip_group_check=True)
                    nc.tensor.matmul(OW, lhsT=pw[m][:, 128:256],
                                     rhs=vh[:, t_, h4, :],
                                     start=(t_ == 0), stop=True,
                                     skip_group_check=True)
                rcp = npool.tile([128, 4], f32, tag="rcp", name="rcp")
                nc.vector.reciprocal(rcp, av[:, 64::65])
                rsg = npool.tile([128, 2], f32, tag="rsg", name="rsg")
                nc.vector.tensor_scalar(rsg, rcp[:, 1::2], float(SG[0]), None, MULT)
                for mq in (0, 1):
                    m = 2 * mh + mq
                    base = 130 * mq
                    tf = npool.tile([128, 64], f32, tag="tf", name="tf")
                    nc.vector.tensor_scalar(tf, av[:, base:base + 64],
                                            rcp[:, 2 * mq:2 * mq + 1], None, MULT)
                    nc.vector.scalar_tensor_tensor(
                        o_chunk[:, m, 64 * h4:64 * h4 + 64],
                        av[:, base + 65:base + 129],
                        rsg[:, mq:mq + 1], tf, MULT, ADD)

        def transp_outproj(c):
            o_chunk = state["oc"]
            for dhalf in (0, 1):
                pst = psPO.tile([128, 512], bf16, tag="po", name="po",
                                  padded_shape=[128, 1024])
                for m in range(4):
                    nc.tensor.matmul(pst[:, ts(m, 128)],
                                     lhsT=o_chunk[:, m, ts(dhalf, 128)],
                                     rhs=idf_sb, is_transpose=True,
                                     start=True, stop=True,
                                     skip_group_check=True)
                nc.vector.tensor_copy(oT[:, dhalf, ts(c, 512)], pst)
            for et in range(8):
                ps = psPO.tile([128, 512], f32, tag="po", name="po")
                for kk in (0, 1):
                    nc.tensor.matmul(ps, lhsT=wo_sb[:, kk, ts(et, 128)],
                                     rhs=oT[:, kk, ts(c, 512)],
                                     start=(kk == 0), stop=(kk == 1))
                ob = opool.tile([128, 512], bf16, tag="ob", name="ob")
                if et % 2 == 0:
                    nc.vector.tensor_copy(ob, ps)
                else:
                    nc.scalar.activation(ob, ps, IDENT)
                nc.sync.dma_start(out=out[ts(et, 128), ts(c, 512)], in_=ob)

        # q/k for chunk 0 start as soon as their weights + x8 arrive
        proj_qk(0)
        msk_sb = cload(msk, [128, 512], bf16, "msk")
        negI_sb = msk_sb[:, 0:128]
        Utri_sb = msk_sb[:, 128:256]
        wm_sb = msk_sb[:, 256:512]
        # remaining (non-critical-path) const loads
        wvT_sb = cload(wvT, [128, KS, 256], bf16, "wvT")
        bvT_blk = consts.tile([128, 256], bf16, tag="bvT", name="bvT")
        nc.sync.dma_start(out=bvT_blk[0:1, :], in_=bvT)
        bvT_sb = bvT_blk[0:1, :]
        ones_blk = consts.tile([128, 128], bf16, tag="ones", name="ones")
        nc.vector.memset(ones_blk, 1.0)
        onesrow = ones_blk[0:1, :]
        wo_sb = cload(wo, [128, 2, DM], bf16, "wo")
        idf_sb = cload(idf, [128, 128], bf16, "idf")
        proj_v(0)
        for c in range(NCH):
            state["oc"] = ochnk.tile([128, 4, 256], bf16, tag="oc", name="oc")
            for h4 in range(4):
                attn(c, h4)
                if h4 == 1 and c + 1 < NCH:
                    proj_v(c + 1)
            if c + 1 < NCH:
                proj_qk(c + 1)
            transp_outproj(c)
        ctx.close()

    nc.compile()
    return nc


def _get_module():
    if "nc" not in _CACHE:
        _CACHE["nc"] = _build_module()
    return _CACHE["nc"]


def _prep_inputs(x, Wq, bq, Wk, bk, Wv, bv, Wo, bo, gate):
    x = np.asarray(x, np.float32)
    Wq = np.asarray(Wq, np.float32)
    Wk = np.asarray(Wk, np.float32)
    Wv = np.asarray(Wv, np.float32)
    Wo = np.asarray(Wo, np.float32)
    bq = np.asarray(bq, np.float32)
    bk = np.asarray(bk, np.float32)
    bv = np.asarray(bv, np.float32)
    SG[0] = float(1.0 / (1.0 + np.exp(-np.float32(gate))))

    fp8_qk = STAGE in ("B2",)
    qknp = FP8 if fp8_qk else BF16
    wscale = 8.0 if fp8_qk else 1.0                       # k weight scale
    qwscale = 8.0 if fp8_qk else 1.0 / np.sqrt(np.float32(DH))
    qbscale = 1.0 / np.sqrt(np.float32(DH))

    j = np.arange(128)[:, None]
    i = np.arange(128)[None, :]
    negI = (-30000.0 * (j == i)).astype(BF16)
    Utri = (j > i).astype(BF16)
    wm_sub = (j >= i + 65).astype(BF16)
    wm_diag = ((j <= i) & (j >= i - 63)).astype(BF16)
    msk = np.concatenate([negI, Utri, wm_sub, wm_diag], axis=1)
    idf = np.eye(128).astype(BF16)

    def wslice(Wmat, e0, scl, dt):
        Ws = (Wmat[e0:e0 + 128, :] * scl).astype(np.float32)  # [128 e', DM d]
        return np.ascontiguousarray(
            Ws.T.reshape(KS, 128, 128).transpose(1, 0, 2)).astype(dt)

    in_maps = []
    for core in range(NCORES):
        b, g = divmod(core, 4)
        e0 = g * 256
        xT_full = np.ascontiguousarray(x[b].T)            # [DM, T]
        xTc = np.ascontiguousarray(
            xT_full.reshape(KS, 128, NCH, 512).transpose(2, 1, 0, 3))
        wq_c = np.stack([wslice(Wq, e0 + 128 * p, qwscale, qknp) for p in (0, 1)])
        wk_c = np.stack([wslice(Wk, e0 + 128 * p, wscale, qknp) for p in (0, 1)])
        Wvs = Wv[e0:e0 + 256, :].astype(np.float32)       # [256 e', DM]
        wvT_c = np.ascontiguousarray(
            Wvs.T.reshape(KS, 128, 256).transpose(1, 0, 2)).astype(BF16)
        Wos = Wo[:, e0:e0 + 256]                          # [DM e, 256 d]
        wo_c = np.ascontiguousarray(
            Wos.T.reshape(2, 128, DM).transpose(1, 0, 2)).astype(BF16)
        bqk_c = np.stack([bq[e0:e0 + 128] * qbscale,
                          bq[e0 + 128:e0 + 256] * qbscale,
                          bk[e0:e0 + 128], bk[e0 + 128:e0 + 256]],
                         axis=1).astype(np.float32)       # [128, 4]
        in_maps.append({
            "xTd": xTc.astype(BF16), "x8d": xTc.astype(qknp),
            "wqk": np.ascontiguousarray(np.stack([wq_c, wk_c]).transpose(2, 0, 1, 3, 4)), "wvT": wvT_c, "wo": wo_c,
            "bqk": bqk_c,
            "bvT": bv[e0:e0 + 256].reshape(1, 256).astype(BF16),
            "msk": msk, "idf": idf,
        })
    return in_maps


def _run(nc, in_maps, **kw):
    from concourse.bass_utils import run_bass_kernel_spmd
    from concourse.bass_interp import get_hw_module
    old = nc.m
    nc.m = get_hw_module(nc.m)
    try:
        res = run_bass_kernel_spmd(nc, in_maps, core_ids=list(range(NCORES)), **kw)
    finally:
        nc.m = old
    return res


def kernel(x, Wq, bq, Wk, bk, Wv, bv, Wo, bo, gate):
    in_maps = _prep_inputs(x, Wq, bq, Wk, bk, Wv, bv, Wo, bo, gate)
    nc = _get_module()
    res = _run(nc, in_maps)
    bo = np.asarray(bo, np.float32)
    out = np.zeros((B, T, DM), np.float32)
    for core in range(NCORES):
        b = core // 4
        out[b] += res.results[core]["out"].astype(np.float32).T
    out += bo[None, None, :]
    return out


# revision 6
# speedup vs baseline: 1.0928x; 1.0637x over previous
"""EvoMultiheadSelfAttention Trainium2 kernel (8 NeuronCores, SPMD), v3.

Sharding: core = (batch b, group of 4 heads). Per core: project q/k/v for
its 4 heads, causal full attention + 64-wide sliding-window attention,
combine with sigmoid(gate), partial output projection over its 256-dim
d-slice. Host sums 4 partials per batch and adds bo.

Key structure:
  - Scores transposed sT[j, i] (keys on partitions); AV produces O[i, d]
    (lhsT = p tiles, rhs = v natural [t, d] + ones lane -> per-partition
    softmax denominators; normalization via per-partition tensor_scalar).
  - v projected DIRECTLY into [t, d] layout (lhsT = xT): no transposes.
  - q/k projections in fp8e4m3 + DoubleRow (2x PE throughput); v path
    stays bf16 (v quantization error dominates otherwise).
  - Causal mask on diag tiles via an extra PE matmul writing -30000 into
    PSUM before the score matmul (group kept contiguous - an intervening
    start=True on a bank wipes open accumulation groups).
  - Bulk score pairs in one 2-bank PSUM tile ([128,1024] exp ops); band
    tiles paired (0,3)/(1,2) into [128,640] exp ops.
  - Window masks multiplied on the Pool engine (SBUF-only, otherwise idle).
  - O transposed to oT[d, t] by PE transpose (f32); outproj per chunk;
    psum->sbuf out copies split DVE/Act; stores via SP DMA.
"""

import os
import numpy as np
import ml_dtypes

B, T, DM, H, WIN = 2, 2048, 1024, 16, 64
DH = DM // H          # 64
NCORES = 8
KS = DM // 128        # 8 d-subtiles
NT = T // 128         # 16 j/i tiles
NCH = T // 512        # 4 chunks of 512
BF16 = ml_dtypes.bfloat16
FP8 = ml_dtypes.float8_e4m3

STAGE = os.environ.get("EVO_STAGE", "B2")

_CACHE: dict = {}
SG = [0.11920292202211755]  # sigmoid(gate); set by _prep_inputs before build


def _build_module():
    import contextlib
    import concourse.bass as bass  # noqa: F401
    import concourse.mybir as mybir
    import concourse.tile as tile
    from concourse import bacc
    from concourse.bass import ts

    f32 = mybir.dt.float32
    bf16 = mybir.dt.bfloat16
    fp8 = mybir.dt.float8e4
    EXP = mybir.ActivationFunctionType.Exp
    IDENT = mybir.ActivationFunctionType.Identity
    MULT = mybir.AluOpType.mult
    ADD = mybir.AluOpType.add
    DR = mybir.MatmulPerfMode.DoubleRow

    fp8_qk = STAGE in ("B2",)
    qkdt = fp8 if fp8_qk else bf16
    qcs = (1.0 / 64.0) if fp8_qk else 1.0   # q copy scale (descale + 1/sqrt(dh))
    kcs = (1.0 / 8.0) if fp8_qk else 1.0    # k copy scale

    nc = bacc.Bacc("TRN2", target_bir_lowering=False, debug=False, num_devices=NCORES)

    def din(name, shape, dt):
        return nc.dram_tensor(name, shape, dt, kind="ExternalInput").ap()

    xTd = din("xTd", [NCH, 128, KS, 512], bf16)    # x[b]^T chunked (bf16, v path)
    x8d = din("x8d", [NCH, 128, KS, 512], qkdt)    # fp8 copy for q/k path
    wqk = din("wqk", [128, 2, 2, KS, 128], qkdt)   # [dp, q/k, p, ks, e']
    wvT = din("wvT", [128, KS, 256], bf16)
    wo = din("wo", [128, 2, DM], bf16)
    bqk = din("bqk", [128, 4], f32)                # [bq p0, bq p1, bk p0, bk p1]
    bvT = din("bvT", [1, 256], bf16)
    # masks packed: [negI | Utri | wm_sub | wm_diag]
    msk = din("msk", [128, 512], bf16)
    idf = din("idf", [128, 128], bf16)             # identity for PE transpose
    out = nc.dram_tensor("out", [DM, T], bf16, kind="ExternalOutput").ap()

    with tile.TileContext(nc) as tc:
        ctx = contextlib.ExitStack()
        consts = ctx.enter_context(tc.tile_pool(name="consts", bufs=1))
        big = ctx.enter_context(tc.tile_pool(name="big", bufs=1))
        pbulk = ctx.enter_context(tc.tile_pool(name="pbulk", bufs=16))
        pband = ctx.enter_context(tc.tile_pool(name="pband", bufs=8))
        ppw = ctx.enter_context(tc.tile_pool(name="ppw", bufs=9))
        npool = ctx.enter_context(tc.tile_pool(name="npool", bufs=6))
        ochnk = ctx.enter_context(tc.tile_pool(name="ochnk", bufs=2))
        opool = ctx.enter_context(tc.tile_pool(name="opool", bufs=6))
        psS = ctx.enter_context(tc.tile_pool(name="psS", bufs=2, space="PSUM"))
        psAV = ctx.enter_context(tc.tile_pool(name="psAV", bufs=2, space="PSUM"))
        psPO = ctx.enter_context(tc.tile_pool(name="psPO", bufs=2, space="PSUM"))

        def cload(ap_in, shape, dt, tag):
            t_ = consts.tile(shape, dt, tag=tag, name=tag)
            nc.sync.dma_start(out=t_, in_=ap_in)
            return t_

        # critical-path loads first: q weights, biases, then x8 chunk 0
        # arrives before k weights / masks (HWDGE+DMA serialize transfers)
        wqk_sb = consts.tile([128, 2, 2, KS, 128], qkdt, tag="wqk", name="wqk")
        nc.sync.dma_start(out=wqk_sb[:, 0], in_=wqk[:, 0])
        bqk_sb = cload(bqk, [128, 4], f32, "bqk")

        warm = consts.tile([128, 1], f32, tag="warm", name="warm")
        nc.vector.memset(warm, 0.0)
        warm2 = consts.tile([128, 1], bf16, tag="warm2", name="warm2")
        nc.scalar.activation(warm2, warm, EXP)

        # xT [dp, ks, t]
        xT = big.tile([128, KS, T], bf16, tag="xT", name="xT")
        x8 = (big.tile([128, KS, T], qkdt, tag="x8", name="x8")
              if fp8_qk else xT)
        qT = [big.tile([128, T], bf16, tag=f"qT{p}", name=f"qT{p}") for p in (0, 1)]
        kT = [big.tile([128, T], bf16, tag=f"kT{p}", name=f"kT{p}") for p in (0, 1)]
        vh = big.tile([128, NT, 4, 65], bf16, tag="vh", name="vh")
        nc.vector.memset(vh, 1.0)
        oT = big.tile([128, 2, T], bf16, tag="oT", name="oT")

        state = {"oc": None}

        def proj_qk(c4):
            if fp8_qk:
                nc.sync.dma_start(out=x8[:, :, ts(c4, 512)], in_=x8d[c4])
            if c4 == 0:
                nc.sync.dma_start(out=wqk_sb[:, 1], in_=wqk[:, 1])
            for p in (0, 1):
                for qk, dst, cs in ((0, qT[p], qcs), (1, kT[p], kcs)):
                    w_sb = wqk_sb[:, qk, p]
                    b_sb = bqk_sb[:, 2 * qk + p:2 * qk + p + 1]
                    ps = psPO.tile([128, 512], f32, tag="po", name="po")
                    if fp8_qk:
                        for kk in range(4):
                            nc.tensor.matmul(ps, lhsT=w_sb[:, 2 * kk:2 * kk + 2, :],
                                             rhs=x8[:, 2 * kk:2 * kk + 2, ts(c4, 512)],
                                             perf_mode=DR,
                                             start=(kk == 0), stop=(kk == 3))
                    else:
                        for kk in range(KS):
                            nc.tensor.matmul(ps, lhsT=w_sb[:, kk, :],
                                             rhs=x8[:, kk, ts(c4, 512)],
                                             start=(kk == 0), stop=(kk == KS - 1))
                    nc.vector.tensor_scalar(dst[:, ts(c4, 512)], ps, cs, b_sb,
                                            MULT, ADD)

        def proj_v(c4):
            nc.sync.dma_start(out=xT[:, :, ts(c4, 512)], in_=xTd[c4])
            for tt in range(4 * c4, 4 * c4 + 4):
                ps = psPO.tile([128, 512], f32, tag="po", name="po")
                pv = ps[:, 0:256]
                nc.tensor.matmul(pv, lhsT=onesrow, rhs=bvT_sb,
                                 start=True, stop=False, skip_group_check=True)
                for kk in range(KS):
                    nc.tensor.matmul(pv, lhsT=xT[:, kk, ts(tt, 128)],
                                     rhs=wvT_sb[:, kk, :],
                                     start=False, stop=(kk == KS - 1),
                                     skip_group_check=True)
                nc.vector.tensor_copy(vh[:, tt, :, 0:64], pv)

        def attn(c, h4):
            """Attention for query chunk c (512 queries), head h4 (0..3)."""
            p, hh = divmod(h4, 2)
            hb = 64 * hh
            kTl, qTl = kT[p], qT[p]
            nbulk = 4 * c
            pa = {}   # bulk pair tiles: pa[jp] covers jt = 2jp, 2jp+1
            for jp in range(nbulk // 2):
                ps = psS.tile([128, 1024], f32, tag="s", name="s")
                for q2 in (0, 1):
                    jt = 2 * jp + q2
                    nc.tensor.matmul(ps[:, ts(q2, 512)],
                                     lhsT=kTl[hb:hb + 64, ts(jt, 128)],
                                     rhs=qTl[hb:hb + 64, ts(c, 512)],
                                     start=True, stop=True,
                                     skip_group_check=(q2 == 1))
                t_ = pbulk.tile([128, 1024], bf16, tag="pa", name="pa")
                nc.scalar.activation(t_, ps, EXP)
                pa[jp] = t_
            # band: subtile m covers j-tiles 4c..4c+m; pairs (0,3), (1,2)
            pb = {}
            for mpair in ((0, 3), (1, 2)):
                ps = psS.tile([128, 1024], f32, tag="s", name="s")
                off = 0
                offs = {}
                for m in mpair:
                    t_ = 4 * c + m
                    for mm in range(m):
                        nc.tensor.matmul(ps[:, off + 128 * mm:off + 128 * mm + 128],
                                         lhsT=kTl[hb:hb + 64, ts(4 * c + mm, 128)],
                                         rhs=qTl[hb:hb + 64, ts(t_, 128)],
                                         start=True, stop=True,
                                         skip_group_check=True)
                    dg = slice(off + 128 * m, off + 128 * m + 128)
                    nc.tensor.matmul(ps[:, dg], lhsT=negI_sb, rhs=Utri_sb,
                                     start=True, stop=False, skip_group_check=True)
                    nc.tensor.matmul(ps[:, dg],
                                     lhsT=kTl[hb:hb + 64, ts(t_, 128)],
                                     rhs=qTl[hb:hb + 64, ts(t_, 128)],
                                     start=False, stop=True,
                                     skip_group_check=True)
                    offs[m] = off
                    off += (m + 1) * 128
                pbt = pband.tile([128, 640], bf16, tag="pb", name="pb")
                nc.scalar.activation(pbt[:, 0:off], ps[:, 0:off], EXP)
                for m in mpair:
                    pb[m] = pbt[:, offs[m]:offs[m] + (m + 1) * 128]
            # window masked probabilities (Pool engine: SBUF-only)
            pw = {}
            for m in range(4):
                t_ = 4 * c + m
                t2 = ppw.tile([128, 256], bf16, tag="pw", name="pw")
                if t_ > 0:
                    if m == 0:
                        sub_src = pa[nbulk // 2 - 1][:, 512:640]
                    else:
                        sub_src = pb[m][:, ts(m - 1, 128)]
                    nc.gpsimd.tensor_tensor(t2[:, 0:128], sub_src,
                                            wm_sb[:, 0:128], MULT)
                nc.vector.tensor_tensor(t2[:, 128:256], pb[m][:, ts(m, 128)],
                                        wm_sb[:, 128:256], MULT)
                pw[m] = t2
            # AV + normalization, two i-subtiles per psum tile
            o_chunk = state["oc"]
            for mh in (0, 1):
                av = psAV.tile([128, 260], f32, tag="av", name="av")
                for mq in (0, 1):
                    m = 2 * mh + mq
                    t_ = 4 * c + m
                    base = 130 * mq
                    OF = av[:, base:base + 65]
                    OW = av[:, base + 65:base + 130]
                    first = True
                    for jt in range(nbulk):
                        nc.tensor.matmul(
                            OF,
                            lhsT=pa[jt // 2][:, 512 * (jt % 2) + 128 * m:
                                             512 * (jt % 2) + 128 * m + 128],
                            rhs=vh[:, jt, h4, :],
                            start=first, stop=False, skip_group_check=True)
                        first = False
                    for mm in range(m + 1):
                        jt = 4 * c + mm
                        nc.tensor.matmul(OF, lhsT=pb[m][:, ts(mm, 128)],
                                         rhs=vh[:, jt, h4, :],
                                         start=first, stop=(mm == m),
                                         skip_group_check=True)
                        first = False
                    if t_ > 0:
                        nc.tensor.matmul(OW, lhsT=pw[m][:, 0:128],
                                         rhs=vh[:, t_ - 1, h4, :],
                                         start=True, stop=False,
                                         skip_group_check=True)
                    nc.tensor.matmul(OW, lhsT=pw[m][:, 128:256],
                                     rhs=vh[:, t_, h4, :],
                                     start=(t_ == 0), stop=True,
                                     skip_group_check=True)
                rcp = npool.tile([128, 4], f32, tag="rcp", name="rcp")
                nc.vector.reciprocal(rcp, av[:, 64::65])
                rsg = npool.tile([128, 2], f32, tag="rsg", name="rsg")
                nc.vector.tensor_scalar(rsg, rcp[:, 1::2], float(SG[0]), None, MULT)
                for mq in (0, 1):
                    m = 2 * mh + mq
                    base = 130 * mq
                    tf = npool.tile([128, 64], f32, tag="tf", name="tf")
                    nc.vector.tensor_scalar(tf, av[:, base:base + 64],
                                            rcp[:, 2 * mq:2 * mq + 1], None, MULT)
                    nc.vector.scalar_tensor_tensor(
                        o_chunk[:, m, 64 * h4:64 * h4 + 64],
                        av[:, base + 65:base + 129],
                        rsg[:, mq:mq + 1], tf, MULT, ADD)

        def transp_outproj(c):
            o_chunk = state["oc"]
            for dhalf in (0, 1):
                pst = psPO.tile([128, 512], bf16, tag="po", name="po",
                                  padded_shape=[128, 1024])
                for m in range(4):
                    nc.tensor.matmul(pst[:, ts(m, 128)],
                                     lhsT=o_chunk[:, m, ts(dhalf, 128)],
                                     rhs=idf_sb, is_transpose=True,
                                     start=True, stop=True,
                                     skip_group_check=True)
                nc.vector.tensor_copy(oT[:, dhalf, ts(c, 512)], pst)
            for et in range(8):
                ps = psPO.tile([128, 512], f32, tag="po", name="po")
                for kk in (0, 1):
                    nc.tensor.matmul(ps, lhsT=wo_sb[:, kk, ts(et, 128)],
                                     rhs=oT[:, kk, ts(c, 512)],
                                     start=(kk == 0), stop=(kk == 1))
                ob = opool.tile([128, 512], bf16, tag="ob", name="ob")
                if et % 2 == 0:
                    nc.vector.tensor_copy(ob, ps)
                else:
                    nc.scalar.activation(ob, ps, IDENT)
                nc.sync.dma_start(out=out[ts(et, 128), ts(c, 512)], in_=ob)

        # q/k for chunk 0 start as soon as their weights + x8 arrive
        proj_qk(0)
        msk_sb = cload(msk, [128, 512], bf16, "msk")
        negI_sb = msk_sb[:, 0:128]
        Utri_sb = msk_sb[:, 128:256]
        wm_sb = msk_sb[:, 256:512]
        # remaining (non-critical-path) const loads
        wvT_sb = cload(wvT, [128, KS, 256], bf16, "wvT")
        bvT_blk = consts.tile([128, 256], bf16, tag="bvT", name="bvT")
        nc.sync.dma_start(out=bvT_blk[0:1, :], in_=bvT)
        bvT_sb = bvT_blk[0:1, :]
        ones_blk = consts.tile([128, 128], bf16, tag="ones", name="ones")
        nc.vector.memset(ones_blk, 1.0)
        onesrow = ones_blk[0:1, :]
        wo_sb = cload(wo, [128, 2, DM], bf16, "wo")
        idf_sb = cload(idf, [128, 128], bf16, "idf")
        proj_v(0)
        for c in range(NCH):
            state["oc"] = ochnk.tile([128, 4, 256], bf16, tag="oc", name="oc")
            for h4 in range(4):
                attn(c, h4)
                if h4 == 1 and c + 1 < NCH:
                    proj_v(c + 1)
            if c + 1 < NCH:
                proj_qk(c + 1)
            transp_outproj(c)
        ctx.close()

    nc.compile()
    return nc


def _get_module():
    if "nc" not in _CACHE:
        _CACHE["nc"] = _build_module()
    return _CACHE["nc"]


def _prep_inputs(x, Wq, bq, Wk, bk, Wv, bv, Wo, bo, gate):
    x = np.asarray(x, np.float32)
    Wq = np.asarray(Wq, np.float32)
    Wk = np.asarray(Wk, np.float32)
    Wv = np.asarray(Wv, np.float32)
    Wo = np.asarray(Wo, np.float32)
    bq = np.asarray(bq, np.float32)
    bk = np.asarray(bk, np.float32)
    bv = np.asarray(bv, np.float32)
    SG[0] = float(1.0 / (1.0 + np.exp(-np.float32(gate))))

    fp8_qk = STAGE in ("B2",)
    qknp = FP8 if fp8_qk else BF16
    wscale = 8.0 if fp8_qk else 1.0                       # k weight scale
    qwscale = 8.0 if fp8_qk else 1.0 / np.sqrt(np.float32(DH))
    qbscale = 1.0 / np.sqrt(np.float32(DH))

    j = np.arange(128)[:, None]
    i = np.arange(128)[None, :]
    negI = (-30000.0 * (j == i)).astype(BF16)
    Utri = (j > i).astype(BF16)
    wm_sub = (j >= i + 65).astype(BF16)
    wm_diag = ((j <= i) & (j >= i - 63)).astype(BF16)
    msk = np.concatenate([negI, Utri, wm_sub, wm_diag], axis=1)
    idf = np.eye(128).astype(BF16)

    def wslice(Wmat, e0, scl, dt):
        Ws = (Wmat[e0:e0 + 128, :] * scl).astype(np.float32)  # [128 e', DM d]
        return np.ascontiguousarray(
            Ws.T.reshape(KS, 128, 128).transpose(1, 0, 2)).astype(dt)

    in_maps = []
    for core in range(NCORES):
        b, g = divmod(core, 4)
        e0 = g * 256
        xT_full = np.ascontiguousarray(x[b].T)            # [DM, T]
        xTc = np.ascontiguousarray(
            xT_full.reshape(KS, 128, NCH, 512).transpose(2, 1, 0, 3))
        wq_c = np.stack([wslice(Wq, e0 + 128 * p, qwscale, qknp) for p in (0, 1)])
        wk_c = np.stack([wslice(Wk, e0 + 128 * p, wscale, qknp) for p in (0, 1)])
        Wvs = Wv[e0:e0 + 256, :].astype(np.float32)       # [256 e', DM]
        wvT_c = np.ascontiguousarray(
            Wvs.T.reshape(KS, 128, 256).transpose(1, 0, 2)).astype(BF16)
        Wos = Wo[:, e0:e0 + 256]                          # [DM e, 256 d]
        wo_c = np.ascontiguousarray(
            Wos.T.reshape(2, 128, DM).transpose(1, 0, 2)).astype(BF16)
        bqk_c = np.stack([bq[e0:e0 + 128] * qbscale,
                          bq[e0 + 128:e0 + 256] * qbscale,
                          bk[e0:e0 + 128], bk[e0 + 128:e0 + 256]],
                         axis=1).astype(np.float32)       # [128, 4]
        in_maps.append({
            "xTd": xTc.astype(BF16), "x8d": xTc.astype(qknp),
            "wqk": np.ascontiguousarray(np.stack([wq_c, wk_c]).transpose(2, 0, 1, 3, 4)), "wvT": wvT_c, "wo": wo_c,
            "bqk": bqk_c,
            "bvT": bv[e0:e0 + 256].reshape(1, 256).astype(BF16),
            "msk": msk, "idf": idf,
        })
    return in_maps


def _run(nc, in_maps, **kw):
    from concourse.bass_utils import run_bass_kernel_spmd
    from concourse.bass_interp import get_hw_module
    old = nc.m
    nc.m = get_hw_module(nc.m)
    try:
        res = run_bass_kernel_spmd(nc, in_maps, core_ids=list(range(NCORES)), **kw)
    finally:
        nc.m = old
    return res


def kernel(x, Wq, bq, Wk, bk, Wv, bv, Wo, bo, gate):
    in_maps = _prep_inputs(x, Wq, bq, Wk, bk, Wv, bv, Wo, bo, gate)
    nc = _get_module()
    res = _run(nc, in_maps)
    bo = np.asarray(bo, np.float32)
    out = np.zeros((B, T, DM), np.float32)
    for core in range(NCORES):
        b = core // 4
        out[b] += res.results[core]["out"].astype(np.float32).T
    out += bo[None, None, :]
    return out


# revision 7
# speedup vs baseline: 1.0993x; 1.0059x over previous
"""EvoMultiheadSelfAttention Trainium2 kernel (8 NeuronCores, SPMD), v3.

Sharding: core = (batch b, group of 4 heads). Per core: project q/k/v for
its 4 heads, causal full attention + 64-wide sliding-window attention,
combine with sigmoid(gate), partial output projection over its 256-dim
d-slice. Host sums 4 partials per batch and adds bo.

Key structure:
  - Scores transposed sT[j, i] (keys on partitions); AV produces O[i, d]
    (lhsT = p tiles, rhs = v natural [t, d] + ones lane -> per-partition
    softmax denominators; normalization via per-partition tensor_scalar).
  - v projected DIRECTLY into [t, d] layout (lhsT = xT): no transposes.
  - q/k projections in fp8e4m3 + DoubleRow (2x PE throughput); v path
    stays bf16 (v quantization error dominates otherwise).
  - Causal mask on diag tiles via an extra PE matmul writing -30000 into
    PSUM before the score matmul (group kept contiguous - an intervening
    start=True on a bank wipes open accumulation groups).
  - Bulk score pairs in one 2-bank PSUM tile ([128,1024] exp ops); band
    tiles paired (0,3)/(1,2) into [128,640] exp ops.
  - Window masks multiplied on the Pool engine (SBUF-only, otherwise idle).
  - O transposed to oT[d, t] by PE transpose (f32); outproj per chunk;
    psum->sbuf out copies split DVE/Act; stores via SP DMA.
"""

import os
import numpy as np
import ml_dtypes

B, T, DM, H, WIN = 2, 2048, 1024, 16, 64
DH = DM // H          # 64
NCORES = 8
KS = DM // 128        # 8 d-subtiles
NT = T // 128         # 16 j/i tiles
NCH = T // 512        # 4 chunks of 512
BF16 = ml_dtypes.bfloat16
FP8 = ml_dtypes.float8_e4m3

STAGE = os.environ.get("EVO_STAGE", "B2")

_CACHE: dict = {}
SG = [0.11920292202211755]  # sigmoid(gate); set by _prep_inputs before build


def _build_module():
    import contextlib
    import concourse.bass as bass  # noqa: F401
    import concourse.mybir as mybir
    import concourse.tile as tile
    from concourse import bacc
    from concourse.bass import ts

    f32 = mybir.dt.float32
    bf16 = mybir.dt.bfloat16
    fp8 = mybir.dt.float8e4
    EXP = mybir.ActivationFunctionType.Exp
    IDENT = mybir.ActivationFunctionType.Identity
    MULT = mybir.AluOpType.mult
    ADD = mybir.AluOpType.add
    DR = mybir.MatmulPerfMode.DoubleRow

    fp8_qk = STAGE in ("B2",)
    qkdt = fp8 if fp8_qk else bf16
    qcs = (1.0 / 64.0) if fp8_qk else 1.0   # q copy scale (descale + 1/sqrt(dh))
    kcs = (1.0 / 8.0) if fp8_qk else 1.0    # k copy scale

    nc = bacc.Bacc("TRN2", target_bir_lowering=False, debug=False, num_devices=NCORES)

    def din(name, shape, dt):
        return nc.dram_tensor(name, shape, dt, kind="ExternalInput").ap()

    xTd = din("xTd", [NCH, 128, KS, 512], bf16)    # x[b]^T chunked (bf16, v path)
    x8d = din("x8d", [NCH, 128, KS, 512], qkdt)    # fp8 copy for q/k path
    wqk = din("wqk", [128, 2, 2, KS, 128], qkdt)   # [dp, q/k, p, ks, e']
    wvT = din("wvT", [128, KS, 256], bf16)
    wo = din("wo", [128, 2, DM], bf16)
    bqk = din("bqk", [128, 4], f32)                # [bq p0, bq p1, bk p0, bk p1]
    bvT = din("bvT", [1, 256], bf16)
    # masks packed: [negI | Utri | wm_sub | wm_diag]
    msk = din("msk", [128, 512], bf16)
    idf = din("idf", [128, 128], bf16)             # identity for PE transpose
    out = nc.dram_tensor("out", [DM, T], bf16, kind="ExternalOutput").ap()

    with tile.TileContext(nc) as tc:
        ctx = contextlib.ExitStack()
        consts = ctx.enter_context(tc.tile_pool(name="consts", bufs=1))
        big = ctx.enter_context(tc.tile_pool(name="big", bufs=1))
        pbulk = ctx.enter_context(tc.tile_pool(name="pbulk", bufs=19))
        pband = ctx.enter_context(tc.tile_pool(name="pband", bufs=11))
        ppw = ctx.enter_context(tc.tile_pool(name="ppw", bufs=13))
        npool = ctx.enter_context(tc.tile_pool(name="npool", bufs=6))
        ochnk = ctx.enter_context(tc.tile_pool(name="ochnk", bufs=2))
        opool = ctx.enter_context(tc.tile_pool(name="opool", bufs=6))
        psS = ctx.enter_context(tc.tile_pool(name="psS", bufs=2, space="PSUM"))
        psAV = ctx.enter_context(tc.tile_pool(name="psAV", bufs=2, space="PSUM"))
        psPO = ctx.enter_context(tc.tile_pool(name="psPO", bufs=2, space="PSUM"))

        def cload(ap_in, shape, dt, tag):
            t_ = consts.tile(shape, dt, tag=tag, name=tag)
            nc.sync.dma_start(out=t_, in_=ap_in)
            return t_

        # critical-path loads first: q weights, biases, then x8 chunk 0
        # arrives before k weights / masks (HWDGE+DMA serialize transfers)
        wqk_sb = consts.tile([128, 2, 2, KS, 128], qkdt, tag="wqk", name="wqk")
        nc.sync.dma_start(out=wqk_sb[:, 0], in_=wqk[:, 0])
        bqk_sb = cload(bqk, [128, 4], f32, "bqk")

        warm = consts.tile([128, 1], f32, tag="warm", name="warm")
        nc.vector.memset(warm, 0.0)
        warm2 = consts.tile([128, 1], bf16, tag="warm2", name="warm2")
        nc.scalar.activation(warm2, warm, EXP)

        # xT [dp, ks, t]
        xT = big.tile([128, KS, T], bf16, tag="xT", name="xT")
        x8 = (big.tile([128, KS, T], qkdt, tag="x8", name="x8")
              if fp8_qk else xT)
        qT = [big.tile([128, T], bf16, tag=f"qT{p}", name=f"qT{p}") for p in (0, 1)]
        kT = [big.tile([128, T], bf16, tag=f"kT{p}", name=f"kT{p}") for p in (0, 1)]
        vh = big.tile([128, NT, 4, 65], bf16, tag="vh", name="vh")
        nc.vector.memset(vh, 1.0)
        oT = big.tile([128, 2, T], bf16, tag="oT", name="oT")

        state = {"oc": None}

        def proj_qk(c4):
            if fp8_qk:
                nc.sync.dma_start(out=x8[:, :, ts(c4, 512)], in_=x8d[c4])
            if c4 == 0:
                nc.sync.dma_start(out=wqk_sb[:, 1], in_=wqk[:, 1])
            for p in (0, 1):
                for qk, dst, cs in ((0, qT[p], qcs), (1, kT[p], kcs)):
                    w_sb = wqk_sb[:, qk, p]
                    b_sb = bqk_sb[:, 2 * qk + p:2 * qk + p + 1]
                    ps = psPO.tile([128, 512], f32, tag="po", name="po")
                    if fp8_qk:
                        for kk in range(4):
                            nc.tensor.matmul(ps, lhsT=w_sb[:, 2 * kk:2 * kk + 2, :],
                                             rhs=x8[:, 2 * kk:2 * kk + 2, ts(c4, 512)],
                                             perf_mode=DR,
                                             start=(kk == 0), stop=(kk == 3))
                    else:
                        for kk in range(KS):
                            nc.tensor.matmul(ps, lhsT=w_sb[:, kk, :],
                                             rhs=x8[:, kk, ts(c4, 512)],
                                             start=(kk == 0), stop=(kk == KS - 1))
                    nc.vector.tensor_scalar(dst[:, ts(c4, 512)], ps, cs, b_sb,
                                            MULT, ADD)

        def proj_v(c4):
            nc.sync.dma_start(out=xT[:, :, ts(c4, 512)], in_=xTd[c4])
            for tt in range(4 * c4, 4 * c4 + 4):
                ps = psPO.tile([128, 512], f32, tag="po", name="po")
                pv = ps[:, 0:256]
                nc.tensor.matmul(pv, lhsT=onesrow, rhs=bvT_sb,
                                 start=True, stop=False, skip_group_check=True)
                for kk in range(KS):
                    nc.tensor.matmul(pv, lhsT=xT[:, kk, ts(tt, 128)],
                                     rhs=wvT_sb[:, kk, :],
                                     start=False, stop=(kk == KS - 1),
                                     skip_group_check=True)
                nc.vector.tensor_copy(vh[:, tt, :, 0:64], pv)

        def attn_scores(c, h4):
            """Scores + exp + window masks for chunk c, head h4."""
            p, hh = divmod(h4, 2)
            hb = 64 * hh
            kTl, qTl = kT[p], qT[p]
            nbulk = 4 * c
            pa = {}   # bulk pair tiles: pa[jp] covers jt = 2jp, 2jp+1
            for jp in range(nbulk // 2):
                ps = psS.tile([128, 1024], f32, tag="s", name="s")
                for q2 in (0, 1):
                    jt = 2 * jp + q2
                    nc.tensor.matmul(ps[:, ts(q2, 512)],
                                     lhsT=kTl[hb:hb + 64, ts(jt, 128)],
                                     rhs=qTl[hb:hb + 64, ts(c, 512)],
                                     start=True, stop=True,
                                     skip_group_check=(q2 == 1))
                t_ = pbulk.tile([128, 1024], bf16, tag="pa", name="pa")
                nc.scalar.activation(t_, ps, EXP)
                pa[jp] = t_
            # band: subtile m covers j-tiles 4c..4c+m; pairs (0,3), (1,2)
            pb = {}
            for mpair in ((0, 3), (1, 2)):
                ps = psS.tile([128, 1024], f32, tag="s", name="s")
                off = 0
                offs = {}
                for m in mpair:
                    t_ = 4 * c + m
                    for mm in range(m):
                        nc.tensor.matmul(ps[:, off + 128 * mm:off + 128 * mm + 128],
                                         lhsT=kTl[hb:hb + 64, ts(4 * c + mm, 128)],
                                         rhs=qTl[hb:hb + 64, ts(t_, 128)],
                                         start=True, stop=True,
                                         skip_group_check=True)
                    dg = slice(off + 128 * m, off + 128 * m + 128)
                    nc.tensor.matmul(ps[:, dg], lhsT=negI_sb, rhs=Utri_sb,
                                     start=True, stop=False, skip_group_check=True)
                    nc.tensor.matmul(ps[:, dg],
                                     lhsT=kTl[hb:hb + 64, ts(t_, 128)],
                                     rhs=qTl[hb:hb + 64, ts(t_, 128)],
                                     start=False, stop=True,
                                     skip_group_check=True)
                    offs[m] = off
                    off += (m + 1) * 128
                pbt = pband.tile([128, 640], bf16, tag="pb", name="pb")
                nc.scalar.activation(pbt[:, 0:off], ps[:, 0:off], EXP)
                for m in mpair:
                    pb[m] = pbt[:, offs[m]:offs[m] + (m + 1) * 128]
            # window masked probabilities (Pool engine: SBUF-only)
            pw = {}
            for m in range(4):
                t_ = 4 * c + m
                t2 = ppw.tile([128, 256], bf16, tag="pw", name="pw")
                if t_ > 0:
                    if m == 0:
                        sub_src = pa[nbulk // 2 - 1][:, 512:640]
                    else:
                        sub_src = pb[m][:, ts(m - 1, 128)]
                    nc.gpsimd.tensor_tensor(t2[:, 0:128], sub_src,
                                            wm_sb[:, 0:128], MULT)
                nc.vector.tensor_tensor(t2[:, 128:256], pb[m][:, ts(m, 128)],
                                        wm_sb[:, 128:256], MULT)
                pw[m] = t2
            return pa, pb, pw

        def attn_av(c, h4, tiles, o_chunk):
            """AV + normalization for chunk c, head h4."""
            pa, pb, pw = tiles
            nbulk = 4 * c
            for mh in (0, 1):
                av = psAV.tile([128, 260], f32, tag="av", name="av")
                for mq in (0, 1):
                    m = 2 * mh + mq
                    t_ = 4 * c + m
                    base = 130 * mq
                    OF = av[:, base:base + 65]
                    OW = av[:, base + 65:base + 130]
                    first = True
                    for jt in range(nbulk):
                        nc.tensor.matmul(
                            OF,
                            lhsT=pa[jt // 2][:, 512 * (jt % 2) + 128 * m:
                                             512 * (jt % 2) + 128 * m + 128],
                            rhs=vh[:, jt, h4, :],
                            start=first, stop=False, skip_group_check=True)
                        first = False
                    for mm in range(m + 1):
                        jt = 4 * c + mm
                        nc.tensor.matmul(OF, lhsT=pb[m][:, ts(mm, 128)],
                                         rhs=vh[:, jt, h4, :],
                                         start=first, stop=(mm == m),
                                         skip_group_check=True)
                        first = False
                    if t_ > 0:
                        nc.tensor.matmul(OW, lhsT=pw[m][:, 0:128],
                                         rhs=vh[:, t_ - 1, h4, :],
                                         start=True, stop=False,
                                         skip_group_check=True)
                    nc.tensor.matmul(OW, lhsT=pw[m][:, 128:256],
                                     rhs=vh[:, t_, h4, :],
                                     start=(t_ == 0), stop=True,
                                     skip_group_check=True)
                rcp = npool.tile([128, 4], f32, tag="rcp", name="rcp")
                nc.vector.reciprocal(rcp, av[:, 64::65])
                rsg = npool.tile([128, 2], f32, tag="rsg", name="rsg")
                nc.vector.tensor_scalar(rsg, rcp[:, 1::2], float(SG[0]), None, MULT)
                for mq in (0, 1):
                    m = 2 * mh + mq
                    base = 130 * mq
                    tf = npool.tile([128, 64], f32, tag="tf", name="tf")
                    nc.vector.tensor_scalar(tf, av[:, base:base + 64],
                                            rcp[:, 2 * mq:2 * mq + 1], None, MULT)
                    nc.vector.scalar_tensor_tensor(
                        o_chunk[:, m, 64 * h4:64 * h4 + 64],
                        av[:, base + 65:base + 129],
                        rsg[:, mq:mq + 1], tf, MULT, ADD)

        def transp_outproj(c):
            o_chunk = state["oc"]
            for dhalf in (0, 1):
                pst = psPO.tile([128, 512], bf16, tag="po", name="po",
                                  padded_shape=[128, 1024])
                for m in range(4):
                    nc.tensor.matmul(pst[:, ts(m, 128)],
                                     lhsT=o_chunk[:, m, ts(dhalf, 128)],
                                     rhs=idf_sb, is_transpose=True,
                                     start=True, stop=True,
                                     skip_group_check=True)
                nc.vector.tensor_copy(oT[:, dhalf, ts(c, 512)], pst)
            for et in range(8):
                ps = psPO.tile([128, 512], f32, tag="po", name="po")
                for kk in (0, 1):
                    nc.tensor.matmul(ps, lhsT=wo_sb[:, kk, ts(et, 128)],
                                     rhs=oT[:, kk, ts(c, 512)],
                                     start=(kk == 0), stop=(kk == 1))
                ob = opool.tile([128, 512], bf16, tag="ob", name="ob")
                if et % 2 == 0:
                    nc.vector.tensor_copy(ob, ps)
                else:
                    nc.scalar.activation(ob, ps, IDENT)
                nc.sync.dma_start(out=out[ts(et, 128), ts(c, 512)], in_=ob)

        # q/k for chunk 0 start as soon as their weights + x8 arrive
        proj_qk(0)
        msk_sb = cload(msk, [128, 512], bf16, "msk")
        negI_sb = msk_sb[:, 0:128]
        Utri_sb = msk_sb[:, 128:256]
        wm_sb = msk_sb[:, 256:512]
        # remaining (non-critical-path) const loads
        wvT_sb = cload(wvT, [128, KS, 256], bf16, "wvT")
        bvT_blk = consts.tile([128, 256], bf16, tag="bvT", name="bvT")
        nc.sync.dma_start(out=bvT_blk[0:1, :], in_=bvT)
        bvT_sb = bvT_blk[0:1, :]
        ones_blk = consts.tile([128, 128], bf16, tag="ones", name="ones")
        nc.vector.memset(ones_blk, 1.0)
        onesrow = ones_blk[0:1, :]
        wo_sb = cload(wo, [128, 2, DM], bf16, "wo")
        idf_sb = cload(idf, [128, 128], bf16, "idf")
        proj_v(0)
        # software-pipelined emission: scores run one head ahead of AV so
        # PE's in-order stream always has independent score work while the
        # Activation engine catches up on the previous head's exps.
        for c in range(NCH):
            state["oc"] = ochnk.tile([128, 4, 256], bf16, tag="oc", name="oc")
            oc = state["oc"]
            t0 = attn_scores(c, 0)
            t1 = attn_scores(c, 1)
            attn_av(c, 0, t0, oc)
            if c + 1 < NCH:
                proj_v(c + 1)
            t2 = attn_scores(c, 2)
            attn_av(c, 1, t1, oc)
            t3 = attn_scores(c, 3)
            if c + 1 < NCH:
                proj_qk(c + 1)
            attn_av(c, 2, t2, oc)
            attn_av(c, 3, t3, oc)
            transp_outproj(c)
        ctx.close()

    nc.compile()
    return nc


def _get_module():
    if "nc" not in _CACHE:
        _CACHE["nc"] = _build_module()
    return _CACHE["nc"]


def _prep_inputs(x, Wq, bq, Wk, bk, Wv, bv, Wo, bo, gate):
    x = np.asarray(x, np.float32)
    Wq = np.asarray(Wq, np.float32)
    Wk = np.asarray(Wk, np.float32)
    Wv = np.asarray(Wv, np.float32)
    Wo = np.asarray(Wo, np.float32)
    bq = np.asarray(bq, np.float32)
    bk = np.asarray(bk, np.float32)
    bv = np.asarray(bv, np.float32)
    SG[0] = float(1.0 / (1.0 + np.exp(-np.float32(gate))))

    fp8_qk = STAGE in ("B2",)
    qknp = FP8 if fp8_qk else BF16
    wscale = 8.0 if fp8_qk else 1.0                       # k weight scale
    qwscale = 8.0 if fp8_qk else 1.0 / np.sqrt(np.float32(DH))
    qbscale = 1.0 / np.sqrt(np.float32(DH))

    j = np.arange(128)[:, None]
    i = np.arange(128)[None, :]
    negI = (-30000.0 * (j == i)).astype(BF16)
    Utri = (j > i).astype(BF16)
    wm_sub = (j >= i + 65).astype(BF16)
    wm_diag = ((j <= i) & (j >= i - 63)).astype(BF16)
    msk = np.concatenate([negI, Utri, wm_sub, wm_diag], axis=1)
    idf = np.eye(128).astype(BF16)

    def wslice(Wmat, e0, scl, dt):
        Ws = (Wmat[e0:e0 + 128, :] * scl).astype(np.float32)  # [128 e', DM d]
        return np.ascontiguousarray(
            Ws.T.reshape(KS, 128, 128).transpose(1, 0, 2)).astype(dt)

    in_maps = []
    for core in range(NCORES):
        b, g = divmod(core, 4)
        e0 = g * 256
        xT_full = np.ascontiguousarray(x[b].T)            # [DM, T]
        xTc = np.ascontiguousarray(
            xT_full.reshape(KS, 128, NCH, 512).transpose(2, 1, 0, 3))
        wq_c = np.stack([wslice(Wq, e0 + 128 * p, qwscale, qknp) for p in (0, 1)])
        wk_c = np.stack([wslice(Wk, e0 + 128 * p, wscale, qknp) for p in (0, 1)])
        Wvs = Wv[e0:e0 + 256, :].astype(np.float32)       # [256 e', DM]
        wvT_c = np.ascontiguousarray(
            Wvs.T.reshape(KS, 128, 256).transpose(1, 0, 2)).astype(BF16)
        Wos = Wo[:, e0:e0 + 256]                          # [DM e, 256 d]
        wo_c = np.ascontiguousarray(
            Wos.T.reshape(2, 128, DM).transpose(1, 0, 2)).astype(BF16)
        bqk_c = np.stack([bq[e0:e0 + 128] * qbscale,
                          bq[e0 + 128:e0 + 256] * qbscale,
                          bk[e0:e0 + 128], bk[e0 + 128:e0 + 256]],
                         axis=1).astype(np.float32)       # [128, 4]
        in_maps.append({
            "xTd": xTc.astype(BF16), "x8d": xTc.astype(qknp),
            "wqk": np.ascontiguousarray(np.stack([wq_c, wk_c]).transpose(2, 0, 1, 3, 4)), "wvT": wvT_c, "wo": wo_c,
            "bqk": bqk_c,
            "bvT": bv[e0:e0 + 256].reshape(1, 256).astype(BF16),
            "msk": msk, "idf": idf,
        })
    return in_maps


def _run(nc, in_maps, **kw):
    from concourse.bass_utils import run_bass_kernel_spmd
    from concourse.bass_interp import get_hw_module
    old = nc.m
    nc.m = get_hw_module(nc.m)
    try:
        res = run_bass_kernel_spmd(nc, in_maps, core_ids=list(range(NCORES)), **kw)
    finally:
        nc.m = old
    return res


def kernel(x, Wq, bq, Wk, bk, Wv, bv, Wo, bo, gate):
    in_maps = _prep_inputs(x, Wq, bq, Wk, bk, Wv, bv, Wo, bo, gate)
    nc = _get_module()
    res = _run(nc, in_maps)
    bo = np.asarray(bo, np.float32)
    out = np.zeros((B, T, DM), np.float32)
    for core in range(NCORES):
        b = core // 4
        out[b] += res.results[core]["out"].astype(np.float32).T
    out += bo[None, None, :]
    return out
